# revision 15
# baseline (speedup 1.0000x reference)
"""Trainium2 Bass kernel for nn_AttentiveEncoderPOS (embed+concat+linear+self-attention).

Sequence-parallel strategy (8 cores, SPMD, AllGather):
  - Core i owns rows i*1024:(i+1)*1024. It gathers ONLY its own 1024 embedding
    rows (bf16 table), computes L_i = concat(emb[ids], pos[pids]) @ W.T + b for
    its slice in BOTH layouts (L.T with h on partitions for the score matmuls,
    and natural V for the AV matmuls), then all-gathers both layouts so every
    core holds the full 8192-row K/V.
  - The position-embedding term folds into the linear as a 64-wide one-hot
    contraction (PW = pos_emb @ W2.T computed on device once); the bias is a
    rank-1 matmul term.  No pos gather, no replicated 8192-row linear.
  - Attention per core: queries = own 1024 rows (SBUF-resident from phase 1),
    keys/values stream from the all-gathered DRAM buffers.  exp(scores) tiles
    for a 512-query half stay in SBUF; AV accumulates over all 64 key tiles in
    single PSUM chains (no SBUF partial-sum accumulation on the vector engine).
  - Scores are tiny (|s|<0.05) so exp() without max-subtraction is exact
    softmax; bf16 matmul inputs, fp32 PSUM accumulation.
  - The AllGathers are chunked (lt/v x first/second half of each rank's key
    tiles) so communication overlaps the start of the attention phase.
"""

import numpy as np
import ml_dtypes

import concourse.bass as bass
import concourse.mybir as mybir
from concourse import bacc
from concourse.tile import TileContext
from concourse.bass_utils import run_bass_kernel_spmd
from concourse.masks import make_identity

N = 8192
H = 1024
VOCAB = 50257
POS = 64
NCORES = 8
NL = N // NCORES          # 1024 rows per core
P = 128
HT = H // P               # 8 h tiles
LRT = NL // P             # 8 local row tiles
KTL = LRT                 # key tiles per rank slab
NCH = 2                   # AG chunks per tensor (kt-local 0:4 / 4:8)
KPC = KTL // NCH          # 4 kt per chunk
KT = N // P               # 64 key tiles global
QH = 2                    # query halves
QW = NL // QH             # 512 queries per half
QT = QW // P              # 4 q tiles per half
HH = 2                    # output h halves
HW = H // HH              # 512
SCALE = 1.0 / 32.0        # 1/sqrt(H)
LAG = 6                   # AV chain lag behind the score/exp pipeline

BF = mybir.dt.bfloat16
F32 = mybir.dt.float32
I32 = mybir.dt.int32
EXP = mybir.ActivationFunctionType.Exp


def build_nc():
    nc = bacc.Bacc(num_devices=NCORES)
    ids = nc.declare_dram_parameter("ids", [LRT, P, 1], I32, isOutput=False)
    emb = nc.declare_dram_parameter("emb", [VOCAB, H], BF, isOutput=False)
    pemb = nc.declare_dram_parameter("pemb", [POS, H], BF, isOutput=False)
    wt = nc.declare_dram_parameter("wt", [2 * H, H], BF, isOutput=False)  # W.T
    oh = nc.declare_dram_parameter("oh", [POS, NL], BF, isOutput=False)  # onehot.T
    brow = nc.declare_dram_parameter("brow", [1, H], BF, isOutput=False)
    out = nc.declare_dram_parameter("out", [NL, H], F32, isOutput=True)

    # AllGather chunk buffers.  lt slab layout: [ktl, p(h in ht), ht, c(key)]
    # so a key tile loads with 2KB-contiguous partition rows; v is natural.
    lt_din = [nc.dram_tensor(f"lt_din{c}", [KPC, P, HT, P], BF) for c in range(NCH)]
    v_din = [nc.dram_tensor(f"v_din{c}", [KPC, P, H], BF) for c in range(NCH)]
    lt_all = [
        nc.dram_tensor(f"lt_all{c}", [NCORES, KPC, P, HT, P], BF, addr_space="Shared")
        for c in range(NCH)
    ]
    v_all = [
        nc.dram_tensor(f"v_all{c}", [NCORES, KPC, P, H], BF, addr_space="Shared")
        for c in range(NCH)
    ]

    with TileContext(nc) as tc:
        with (
            tc.tile_pool(name="const", bufs=1) as const,
            tc.tile_pool(name="qtp", bufs=HT) as qtp,
        ):
            ident = const.tile([P, P], BF)
            make_identity(nc, ident[:])
            ident32 = const.tile([P, P], F32)
            make_identity(nc, ident32[:])
            ones_row = const.tile([1, NL], BF)
            nc.gpsimd.memset(ones_row[:], 1.0)
            ones_col = const.tile([P, 1], BF)
            nc.gpsimd.memset(ones_col[:], 1.0)
            b_row = const.tile([1, H], BF)
            nc.sync.dma_start(out=b_row[:], in_=brow[:, :])
            oh_sb = const.tile([POS, NL], BF)
            nc.sync.dma_start(out=oh_sb[:], in_=oh[:, :])
            pw = const.tile([POS, H], BF)
            # query-side L.T (own rows), kept resident for phase 2
            qts = [qtp.tile([P, NL], BF, tag="q", name="q") for _ in range(HT)]

            # ---------------- Phase 1: own-slice linear ----------------
            with (
                tc.tile_pool(name="wtp", bufs=1) as wtp,
                tc.tile_pool(name="idp", bufs=LRT) as idp,
                tc.tile_pool(name="xfp", bufs=LRT) as xfp,
                tc.tile_pool(name="xtp", bufs=1) as xtp,
                tc.tile_pool(name="pep", bufs=HT + 1) as pep,
                tc.tile_pool(name="vsp", bufs=LRT) as vsp,
                tc.tile_pool(name="tps", bufs=2, space="PSUM") as tps,
                tc.tile_pool(name="mps", bufs=3, space="PSUM") as mps,
                tc.tile_pool(name="pps", bufs=2, space="PSUM") as pps,
            ):
                # own embedding row gather first (SWDGE; overlaps the W loads)
                xfs = []
                for rt in range(LRT):
                    idt = idp.tile([P, 1], I32, tag="id", name="id")
                    nc.sync.dma_start(out=idt[:], in_=ids[rt])
                    xf = xfp.tile([P, H], BF, tag="xf", name="xf")
                    nc.gpsimd.indirect_dma_start(
                        out=xf[:],
                        out_offset=None,
                        in_=emb[:],
                        in_offset=bass.IndirectOffsetOnAxis(ap=idt[:, :1], axis=0),
                    )
                    xfs.append(xf)

                # W.T bf16 -> SBUF: one strided DMA, partition p holds all 16
                # contraction blocks' row p
                wtb_all = wtp.tile([P, 2 * HT, H], BF, name="wtall")
                nc.scalar.dma_start(
                    out=wtb_all[:],
                    in_=wt.rearrange("(k p) h -> p k h", p=P),
                )
                wtb = [wtb_all[:, k, :] for k in range(2 * HT)]

                # PW = pemb @ W2.T  (pos table projected once)
                pe_sb = pep.tile([POS, H], BF, tag="pe", name="pe")
                nc.sync.dma_start(out=pe_sb[:], in_=pemb[:, :])
                pets = []
                for jt in range(HT):
                    pt = tps.tile([P, POS], BF, tag="xt", name="pet")
                    nc.tensor.transpose(
                        pt[:], pe_sb[:, jt * P : (jt + 1) * P], ident[0:POS, 0:POS]
                    )
                    pe_t = pep.tile([P, POS], BF, tag="pett", name="pett")
                    nc.vector.tensor_copy(out=pe_t[:], in_=pt[:])
                    pets.append(pe_t)
                for hh in range(HH):
                    pm = pps.tile([POS, HW], F32, tag="pwp", name="pwp")
                    for jt in range(HT):
                        nc.tensor.matmul(
                            pm[:],
                            lhsT=pets[jt][:],
                            rhs=wtb[HT + jt][:, hh * HW : (hh + 1) * HW],
                            start=(jt == 0),
                            stop=(jt == HT - 1),
                        )
                    nc.vector.tensor_copy(out=pw[:, hh * HW : (hh + 1) * HW], in_=pm[:])

                # transpose gathered rows to X.T
                # xts free layout: k*NL + rt*P + rr
                xts = xtp.tile([P, HT * NL], BF, name="xts")
                for rt in range(LRT):
                    pt = tps.tile([P, H], BF, tag="xt", name="xt")
                    for k in range(HT):
                        nc.tensor.transpose(
                            pt[:, k * P : (k + 1) * P],
                            xfs[rt][:, k * P : (k + 1) * P],
                            ident[:],
                        )
                    nc.vector.tensor_copy(
                        out=xts[:]
                        .rearrange("p (k rt rr) -> p k rt rr", k=HT, rt=LRT)[
                            :, :, rt, :
                        ],
                        in_=pt[:].rearrange("p (k rr) -> p k rr", k=HT),
                    )

                def lt_chunk(ch):
                    for ht in range(HT):
                        pm = mps.tile([P, QW], F32, tag="mm", name="mm")
                        for k in range(HT):
                            nc.tensor.matmul(
                                pm[:],
                                lhsT=wtb[k][:, ht * P : (ht + 1) * P],
                                rhs=xts[:, k * NL + ch * QW : k * NL + (ch + 1) * QW],
                                start=(k == 0),
                                stop=False,
                            )
                        nc.tensor.matmul(
                            pm[:],
                            lhsT=pw[:, ht * P : (ht + 1) * P],
                            rhs=oh_sb[:, ch * QW : (ch + 1) * QW],
                            start=False,
                            stop=False,
                        )
                        nc.tensor.matmul(
                            pm[:],
                            lhsT=b_row[0:1, ht * P : (ht + 1) * P],
                            rhs=ones_row[0:1, ch * QW : (ch + 1) * QW],
                            start=False,
                            stop=True,
                        )
                        nc.vector.tensor_copy(
                            out=qts[ht][:, ch * QW : (ch + 1) * QW], in_=pm[:]
                        )
                        nc.sync.dma_start(
                            out=lt_din[ch][:, :, ht, :].rearrange("kb p c -> p kb c"),
                            in_=qts[ht][:, ch * QW : (ch + 1) * QW].rearrange(
                                "p (kb c) -> p kb c", kb=KPC
                            ),
                        )

                def v_chunk(ch):
                    # V rows = transposes of the finished L.T columns
                    for rt in range(ch * KPC, (ch + 1) * KPC):
                        pt = tps.tile([P, H], BF, tag="xt", name="vt")
                        for ht in range(HT):
                            nc.tensor.transpose(
                                pt[:, ht * P : (ht + 1) * P],
                                qts[ht][:, rt * P : (rt + 1) * P],
                                ident[:],
                            )
                        v_sb = vsp.tile([P, H], BF, tag="vsb", name="vsb")
                        nc.vector.tensor_copy(out=v_sb[:], in_=pt[:])
                        nc.sync.dma_start(out=v_din[rt // KPC][rt % KPC], in_=v_sb[:])

                lt_chunk(0)
                v_chunk(0)
                lt_chunk(1)
                v_chunk(1)

            # ---------------- AllGathers (chunked) ----------------
            for c in range(NCH):
                nc.gpsimd.collective_compute(
                    "AllGather",
                    mybir.AluOpType.bypass,
                    replica_groups=[list(range(NCORES))],
                    ins=[lt_din[c][:, :, :, :]],
                    outs=[lt_all[c][:, :, :, :, :]],
                )
                nc.gpsimd.collective_compute(
                    "AllGather",
                    mybir.AluOpType.bypass,
                    replica_groups=[list(range(NCORES))],
                    ins=[v_din[c][:, :, :]],
                    outs=[v_all[c][:, :, :, :]],
                )

            # ---------------- Phase 2: attention ----------------
            kts = [(c, r, k) for c in range(NCH) for r in range(NCORES) for k in range(KPC)]
            with (
                tc.tile_pool(name="lkp", bufs=6) as lkp,
                tc.tile_pool(name="vp2", bufs=LAG + 4) as vp2,
                tc.tile_pool(name="ep", bufs=KT) as ep,
                tc.tile_pool(name="fin", bufs=4) as fin,
                tc.tile_pool(name="osb", bufs=4) as osb,
                tc.tile_pool(name="sps", bufs=2, space="PSUM") as sps,
                tc.tile_pool(name="ops", bufs=QT, space="PSUM") as ops,
                tc.tile_pool(name="cps", bufs=1, space="PSUM") as cps,
                tc.tile_pool(name="rps", bufs=1, space="PSUM") as rps,
            ):
                rec_ps = rps.tile([P, QH * QT], F32, name="recp")
                for qh in range(QH):
                    qsl = slice(qh * QW, (qh + 1) * QW)
                    psum_cs = cps.tile([1, QW], F32, tag="cs", name="cs")
                    es = []
                    vts = []
                    pos_ = [
                        ops.tile([P, HW], F32, tag="po", name="po") for _ in range(QT)
                    ]

                    def av0(ki):
                        nc.tensor.matmul(
                            psum_cs[0:1, :],
                            lhsT=ones_col[:],
                            rhs=es[ki][:],
                            start=(ki == 0),
                            stop=(ki == KT - 1),
                        )
                        for qt in range(QT):
                            nc.tensor.matmul(
                                pos_[qt][:],
                                lhsT=es[ki][:, qt * P : (qt + 1) * P],
                                rhs=vts[ki][:],
                                start=(ki == 0),
                                stop=(ki == KT - 1),
                            )

                    # stage A: scores+exp; colsum/AV-hh0 run one kt behind so
                    # the exp latency hides under the next score chain
                    for ki, (c, r, ktl) in enumerate(kts):
                        ltk = lkp.tile([P, HT * P], BF, tag="lk", name="lk")
                        nc.sync.dma_start(
                            out=ltk[:].rearrange("p (h c) -> p h c", h=HT),
                            in_=lt_all[c][r, ktl],
                        )
                        ps = sps.tile([P, QW], F32, tag="sp", name="sp")
                        for ht in range(HT):
                            nc.tensor.matmul(
                                ps[:],
                                lhsT=ltk[:, ht * P : (ht + 1) * P],
                                rhs=qts[ht][:, qsl],
                                start=(ht == 0),
                                stop=(ht == HT - 1),
                            )
                        e = ep.tile([P, QW], BF, tag="e", name="e")
                        nc.scalar.activation(out=e[:], in_=ps[:], func=EXP, scale=SCALE)
                        es.append(e)
                        vt = vp2.tile([P, HW], BF, tag="v2", name="v2")
                        nc.scalar.dma_start(out=vt[:], in_=v_all[c][r, ktl][:, 0:HW])
                        vts.append(vt)
                        if ki >= LAG:
                            av0(ki - LAG)
                    for j in range(KT - LAG, KT):
                        av0(j)

                    # denominator -> reciprocal per q tile
                    cs_sb = fin.tile([1, QW], F32, tag="cs_sb", name="cs_sb")
                    nc.vector.tensor_copy(out=cs_sb[:], in_=psum_cs[0:1, :])
                    for qt in range(QT):
                        nc.tensor.transpose(
                            rec_ps[:, qh * QT + qt : qh * QT + qt + 1],
                            cs_sb[0:1, qt * P : (qt + 1) * P],
                            ident32[0:1, 0:1],
                        )
                    rec = fin.tile([P, QT], F32, tag="rec", name="rec")
                    nc.vector.reciprocal(rec[:], rec_ps[:, qh * QT : (qh + 1) * QT])

                    # finalize hh0
                    for qt in range(QT):
                        o_sb = osb.tile([P, HW], F32, tag="o", name="o")
                        nc.vector.tensor_mul(
                            out=o_sb[:],
                            in0=pos_[qt][:],
                            in1=rec[:, qt : qt + 1].to_broadcast([P, HW]),
                        )
                        nc.sync.dma_start(
                            out=out[qh * QW + qt * P : qh * QW + (qt + 1) * P, 0:HW],
                            in_=o_sb[:],
                        )

                    # hh1 chains (reuse es, second half of V)
                    pos1 = [
                        ops.tile([P, HW], F32, tag="po", name="po") for _ in range(QT)
                    ]
                    for ki, (c, r, ktl) in enumerate(kts):
                        vt = vp2.tile([P, HW], BF, tag="v2", name="v2")
                        nc.sync.dma_start(out=vt[:], in_=v_all[c][r, ktl][:, HW:H])
                        for qt in range(QT):
                            nc.tensor.matmul(
                                pos1[qt][:],
                                lhsT=es[ki][:, qt * P : (qt + 1) * P],
                                rhs=vt[:],
                                start=(ki == 0),
                                stop=(ki == KT - 1),
                            )
                    for qt in range(QT):
                        o_sb = osb.tile([P, HW], F32, tag="o", name="o")
                        nc.vector.tensor_mul(
                            out=o_sb[:],
                            in0=pos1[qt][:],
                            in1=rec[:, qt : qt + 1].to_broadcast([P, HW]),
                        )
                        nc.sync.dma_start(
                            out=out[qh * QW + qt * P : qh * QW + (qt + 1) * P, HW:H],
                            in_=o_sb[:],
                        )
    nc.finalize()
    return nc


def _prep_inputs(inputs):
    ids = np.asarray(inputs["input_ids"]).astype(np.int32)
    pids = np.asarray(inputs["pos_ids"]).astype(np.int32)
    emb = np.asarray(inputs["emb"], dtype=np.float32).astype(ml_dtypes.bfloat16)
    pemb = np.asarray(inputs["pos_emb"], dtype=np.float32).astype(ml_dtypes.bfloat16)
    W = np.asarray(inputs["W"], dtype=np.float32)
    b = np.asarray(inputs["b"], dtype=np.float32)
    wt = np.ascontiguousarray(W.T).astype(ml_dtypes.bfloat16)  # [2H, H]
    brow = b.reshape(1, H).astype(ml_dtypes.bfloat16)
    in_maps = []
    for i in range(NCORES):
        sl = slice(i * NL, (i + 1) * NL)
        oh = np.zeros((POS, NL), dtype=ml_dtypes.bfloat16)
        oh[pids[sl], np.arange(NL)] = 1.0
        in_maps.append(
            {
                "ids": np.ascontiguousarray(ids[sl].reshape(LRT, P, 1)),
                "emb": emb,
                "pemb": pemb,
                "wt": wt,
                "oh": oh,
                "brow": brow,
            }
        )
    return in_maps


def run(inputs, trace=False):
    nc = build_nc()
    in_maps = _prep_inputs(inputs)
    res = run_bass_kernel_spmd(nc, in_maps, list(range(NCORES)), trace=trace)
    out = np.concatenate([res.results[i]["out"] for i in range(NCORES)], axis=0)
    return out, res


def kernel(**inputs):
    out, _ = run(inputs, trace=False)
    return out


# revision 16
# speedup vs baseline: 1.2807x; 1.2807x over previous
"""Trainium2 Bass kernel for nn_AttentiveEncoderPOS (embed+concat+linear+self-attention).

Sequence-parallel strategy (8 cores, SPMD, AllGather):
  - Core i owns rows i*1024:(i+1)*1024. It gathers ONLY its own 1024 embedding
    rows (bf16 table), computes L_i = concat(emb[ids], pos[pids]) @ W.T + b for
    its slice in BOTH layouts (L.T with h on partitions for the score matmuls,
    and natural V for the AV matmuls), then all-gathers both layouts so every
    core holds the full 8192-row K/V.
  - The position-embedding term folds into the linear as a 64-wide one-hot
    contraction (PW = pos_emb @ W2.T computed on device once); the bias is a
    rank-1 matmul term.  No pos gather, no replicated 8192-row linear.
  - Attention per core: queries = own 1024 rows (SBUF-resident from phase 1),
    keys/values stream from the all-gathered DRAM buffers.  exp(scores) tiles
    for a 512-query half stay in SBUF; AV accumulates over all 64 key tiles in
    single PSUM chains (no SBUF partial-sum accumulation on the vector engine).
  - Scores are tiny (|s|<0.05) so exp() without max-subtraction is exact
    softmax; bf16 matmul inputs, fp32 PSUM accumulation.
  - The AllGathers are chunked (lt/v x first/second half of each rank's key
    tiles) so communication overlaps the start of the attention phase.
"""

import numpy as np
import ml_dtypes

import concourse.bass as bass
import concourse.mybir as mybir
from concourse import bacc
from concourse.tile import TileContext
from concourse.bass_utils import run_bass_kernel_spmd
from concourse.masks import make_identity

N = 8192
H = 1024
VOCAB = 50257
POS = 64
NCORES = 8
NL = N // NCORES          # 1024 rows per core
P = 128
HT = H // P               # 8 h tiles
LRT = NL // P             # 8 local row tiles
KTL = LRT                 # key tiles per rank slab
NCH = 2                   # AG chunks per tensor (kt-local 0:4 / 4:8)
KPC = KTL // NCH          # 4 kt per chunk
KT = N // P               # 64 key tiles global
QH = 2                    # query halves
QW = NL // QH             # 512 queries per half
QT = QW // P              # 4 q tiles per half
HH = 2                    # output h halves
HW = H // HH              # 512
SCALE = 1.0 / 512.0       # 1/sqrt(H) / 16 (L carried x4 for fp8 range)
LAG = 6                   # AV chain lag behind the score/exp pipeline

BF = mybir.dt.bfloat16
F32 = mybir.dt.float32
I32 = mybir.dt.int32
EXP = mybir.ActivationFunctionType.Exp
F8 = mybir.dt.float8e4


def build_nc():
    nc = bacc.Bacc(num_devices=NCORES)
    ids = nc.declare_dram_parameter("ids", [LRT, P, 1], I32, isOutput=False)
    emb = nc.declare_dram_parameter("emb", [VOCAB, H], BF, isOutput=False)
    pemb = nc.declare_dram_parameter("pemb", [POS, H], BF, isOutput=False)
    wt = nc.declare_dram_parameter("wt", [P, 2 * HT, H], BF, isOutput=False)  # W.T swizzled
    oh = nc.declare_dram_parameter("oh", [POS, NL], BF, isOutput=False)  # onehot.T
    brow = nc.declare_dram_parameter("brow", [1, H], BF, isOutput=False)
    out = nc.declare_dram_parameter("out", [NL, H], F32, isOutput=True)

    # AllGather chunk buffers.  lt slab layout: [ktl, p(h in ht), ht, c(key)]
    # so a key tile loads with 2KB-contiguous partition rows; v is natural.
    lt_din = [nc.dram_tensor(f"lt_din{c}", [KPC, P, HT, P], F8) for c in range(NCH)]
    v_din = [nc.dram_tensor(f"v_din{c}", [KPC, P, H], BF) for c in range(NCH)]
    lt_all = [
        nc.dram_tensor(f"lt_all{c}", [NCORES, KPC, P, HT, P], F8, addr_space="Shared")
        for c in range(NCH)
    ]
    v_all = [
        nc.dram_tensor(f"v_all{c}", [NCORES, KPC, P, H], BF, addr_space="Shared")
        for c in range(NCH)
    ]

    with TileContext(nc) as tc:
        with (
            tc.tile_pool(name="const", bufs=1) as const,
            tc.tile_pool(name="qtp", bufs=HT) as qtp,
        ):
            ident = const.tile([P, P], BF)
            make_identity(nc, ident[:])
            ident32 = const.tile([P, P], F32)
            make_identity(nc, ident32[:])
            ones_row = const.tile([1, NL], BF)
            nc.gpsimd.memset(ones_row[:], 1.0)
            fours_col = const.tile([P, 1], BF)
            nc.gpsimd.memset(fours_col[:], 4.0)
            b_row = const.tile([1, H], BF)
            nc.sync.dma_start(out=b_row[:], in_=brow[:, :])
            oh_sb = const.tile([POS, NL], BF)
            nc.sync.dma_start(out=oh_sb[:], in_=oh[:, :])
            pw = const.tile([POS, H], BF)
            qts8 = const.tile([P, HT, NL], F8)
            # query-side L.T (own rows), kept resident for phase 2
            qts = [qtp.tile([P, NL], BF, tag="q", name="q") for _ in range(HT)]

            # ---------------- Phase 1: own-slice linear ----------------
            with (
                tc.tile_pool(name="wtp", bufs=1) as wtp,
                tc.tile_pool(name="idp", bufs=LRT) as idp,
                tc.tile_pool(name="xfp", bufs=LRT) as xfp,
                tc.tile_pool(name="xtp", bufs=1) as xtp,
                tc.tile_pool(name="pep", bufs=HT + 1) as pep,
                tc.tile_pool(name="vsp", bufs=LRT) as vsp,
                tc.tile_pool(name="tps", bufs=2, space="PSUM") as tps,
                tc.tile_pool(name="mps", bufs=3, space="PSUM") as mps,
                tc.tile_pool(name="pps", bufs=2, space="PSUM") as pps,
            ):
                # own embedding row gather first (SWDGE; overlaps the W loads)
                xfs = []
                for rt in range(LRT):
                    idt = idp.tile([P, 1], I32, tag="id", name="id")
                    nc.sync.dma_start(out=idt[:], in_=ids[rt])
                    xf = xfp.tile([P, H], BF, tag="xf", name="xf")
                    nc.gpsimd.indirect_dma_start(
                        out=xf[:],
                        out_offset=None,
                        in_=emb[:],
                        in_offset=bass.IndirectOffsetOnAxis(ap=idt[:, :1], axis=0),
                    )
                    xfs.append(xf)

                # W.T bf16 -> SBUF: one strided DMA, partition p holds all 16
                # contraction blocks' row p
                wtb_all = wtp.tile([P, 2 * HT, H], BF, name="wtall")
                nc.scalar.dma_start(out=wtb_all[:], in_=wt[:, :, :])
                wtb = [wtb_all[:, k, :] for k in range(2 * HT)]

                # transpose gathered rows to X.T
                # xts free layout: k*NL + rt*P + rr
                xts = xtp.tile([P, HT * NL], BF, name="xts")
                for rt in range(LRT):
                    pt = tps.tile([P, H], BF, tag="xt", name="xt")
                    for k in range(HT):
                        nc.tensor.transpose(
                            pt[:, k * P : (k + 1) * P],
                            xfs[rt][:, k * P : (k + 1) * P],
                            ident[:],
                        )
                    nc.vector.tensor_copy(
                        out=xts[:]
                        .rearrange("p (k rt rr) -> p k rt rr", k=HT, rt=LRT)[
                            :, :, rt, :
                        ],
                        in_=pt[:].rearrange("p (k rr) -> p k rr", k=HT),
                    )

                # PW = pemb @ W2.T  (pos table projected once)
                pe_sb = pep.tile([POS, H], BF, tag="pe", name="pe")
                nc.sync.dma_start(out=pe_sb[:], in_=pemb[:, :])
                pets = []
                for jt in range(HT):
                    pt = tps.tile([P, POS], BF, tag="xt", name="pet")
                    nc.tensor.transpose(
                        pt[:], pe_sb[:, jt * P : (jt + 1) * P], ident[0:POS, 0:POS]
                    )
                    pe_t = pep.tile([P, POS], BF, tag="pett", name="pett")
                    nc.vector.tensor_copy(out=pe_t[:], in_=pt[:])
                    pets.append(pe_t)
                for hh in range(HH):
                    pm = pps.tile([POS, HW], F32, tag="pwp", name="pwp")
                    for jt in range(HT):
                        nc.tensor.matmul(
                            pm[:],
                            lhsT=pets[jt][:],
                            rhs=wtb[HT + jt][:, hh * HW : (hh + 1) * HW],
                            start=(jt == 0),
                            stop=(jt == HT - 1),
                        )
                    nc.vector.tensor_copy(out=pw[:, hh * HW : (hh + 1) * HW], in_=pm[:])

                def lt_chunk(ch):
                    for ht in range(HT):
                        pm = mps.tile([P, QW], F32, tag="mm", name="mm")
                        for k in range(HT):
                            nc.tensor.matmul(
                                pm[:],
                                lhsT=wtb[k][:, ht * P : (ht + 1) * P],
                                rhs=xts[:, k * NL + ch * QW : k * NL + (ch + 1) * QW],
                                start=(k == 0),
                                stop=False,
                            )
                        nc.tensor.matmul(
                            pm[:],
                            lhsT=pw[:, ht * P : (ht + 1) * P],
                            rhs=oh_sb[:, ch * QW : (ch + 1) * QW],
                            start=False,
                            stop=False,
                        )
                        nc.tensor.matmul(
                            pm[:],
                            lhsT=b_row[0:1, ht * P : (ht + 1) * P],
                            rhs=ones_row[0:1, ch * QW : (ch + 1) * QW],
                            start=False,
                            stop=True,
                        )
                        nc.vector.tensor_copy(
                            out=qts[ht][:, ch * QW : (ch + 1) * QW], in_=pm[:]
                        )
                        nc.vector.tensor_copy(
                            out=qts8[:, ht, ch * QW : (ch + 1) * QW], in_=pm[:]
                        )
                        nc.sync.dma_start(
                            out=lt_din[ch][:, :, ht, :].rearrange("kb p c -> p kb c"),
                            in_=qts8[:, ht, ch * QW : (ch + 1) * QW].rearrange(
                                "p (kb c) -> p kb c", kb=KPC
                            ),
                        )

                def v_chunk(ch):
                    # V rows = transposes of the finished L.T columns
                    for rt in range(ch * KPC, (ch + 1) * KPC):
                        pt = tps.tile([P, H], BF, tag="xt", name="vt")
                        for ht in range(HT):
                            nc.tensor.transpose(
                                pt[:, ht * P : (ht + 1) * P],
                                qts[ht][:, rt * P : (rt + 1) * P],
                                ident[:],
                            )
                        v_sb = vsp.tile([P, H], BF, tag="vsb", name="vsb")
                        nc.vector.tensor_copy(out=v_sb[:], in_=pt[:])
                        nc.sync.dma_start(out=v_din[rt // KPC][rt % KPC], in_=v_sb[:])

                lt_chunk(0)
                v_chunk(0)
                lt_chunk(1)
                v_chunk(1)

            # ---------------- AllGathers (chunked) ----------------
            for c in range(NCH):
                nc.gpsimd.collective_compute(
                    "AllGather",
                    mybir.AluOpType.bypass,
                    replica_groups=[list(range(NCORES))],
                    ins=[lt_din[c][:, :, :, :]],
                    outs=[lt_all[c][:, :, :, :, :]],
                )
                nc.gpsimd.collective_compute(
                    "AllGather",
                    mybir.AluOpType.bypass,
                    replica_groups=[list(range(NCORES))],
                    ins=[v_din[c][:, :, :]],
                    outs=[v_all[c][:, :, :, :]],
                )

            # ---------------- Phase 2: attention ----------------
            kts = [(c, r, k) for c in range(NCH) for r in range(NCORES) for k in range(KPC)]
            with (
                tc.tile_pool(name="lkp", bufs=6) as lkp,
                tc.tile_pool(name="vp2", bufs=LAG + 4) as vp2,
                tc.tile_pool(name="ep", bufs=KT) as ep,
                tc.tile_pool(name="fin", bufs=4) as fin,
                tc.tile_pool(name="osb", bufs=4) as osb,
                tc.tile_pool(name="sps", bufs=2, space="PSUM") as sps,
                tc.tile_pool(name="ops", bufs=QT, space="PSUM") as ops,
                tc.tile_pool(name="cps", bufs=1, space="PSUM") as cps,
                tc.tile_pool(name="rps", bufs=1, space="PSUM") as rps,
            ):
                rec_ps = rps.tile([P, QH * QT], F32, name="recp")
                for qh in range(QH):
                    qsl = slice(qh * QW, (qh + 1) * QW)
                    psum_cs = cps.tile([1, QW], F32, tag="cs", name="cs")
                    es = []
                    vts = []
                    pos_ = [
                        ops.tile([P, HW], F32, tag="po", name="po") for _ in range(QT)
                    ]

                    def av0(ki):
                        nc.tensor.matmul(
                            psum_cs[0:1, :],
                            lhsT=fours_col[:],
                            rhs=es[ki][:],
                            start=(ki == 0),
                            stop=(ki == KT - 1),
                        )
                        for qt in range(QT):
                            nc.tensor.matmul(
                                pos_[qt][:],
                                lhsT=es[ki][:, qt * P : (qt + 1) * P],
                                rhs=vts[ki][:],
                                start=(ki == 0),
                                stop=(ki == KT - 1),
                            )

                    # stage A: scores+exp; colsum/AV-hh0 run one kt behind so
                    # the exp latency hides under the next score chain
                    for ki, (c, r, ktl) in enumerate(kts):
                        ltk = lkp.tile([P, HT * P], F8, tag="lk", name="lk")
                        nc.sync.dma_start(
                            out=ltk[:].rearrange("p (h c) -> p h c", h=HT),
                            in_=lt_all[c][r, ktl],
                        )
                        ltk3 = ltk[:].rearrange("p (h c) -> p h c", h=HT)
                        ps = sps.tile([P, QW], F32, tag="sp", name="sp")
                        for t in range(HT // 2):
                            nc.tensor.matmul(
                                ps[:],
                                lhsT=ltk3[:, 2 * t : 2 * t + 2, :],
                                rhs=qts8[:, 2 * t : 2 * t + 2, qsl],
                                start=(t == 0),
                                stop=(t == HT // 2 - 1),
                                perf_mode=mybir.MatmulPerfMode.DoubleRow,
                            )
                        e = ep.tile([P, QW], BF, tag="e", name="e")
                        nc.scalar.activation(out=e[:], in_=ps[:], func=EXP, scale=SCALE)
                        es.append(e)
                        vt = vp2.tile([P, HW], BF, tag="v2", name="v2")
                        nc.sync.dma_start(out=vt[:], in_=v_all[c][r, ktl][:, 0:HW])
                        vts.append(vt)
                        if ki >= LAG:
                            av0(ki - LAG)
                    for j in range(KT - LAG, KT):
                        av0(j)

                    # denominator -> reciprocal per q tile
                    cs_sb = fin.tile([1, QW], F32, tag="cs_sb", name="cs_sb")
                    nc.vector.tensor_copy(out=cs_sb[:], in_=psum_cs[0:1, :])
                    for qt in range(QT):
                        nc.tensor.transpose(
                            rec_ps[:, qh * QT + qt : qh * QT + qt + 1],
                            cs_sb[0:1, qt * P : (qt + 1) * P],
                            ident32[0:1, 0:1],
                        )
                    rec = fin.tile([P, QT], F32, tag="rec", name="rec")
                    nc.vector.reciprocal(rec[:], rec_ps[:, qh * QT : (qh + 1) * QT])

                    # finalize hh0
                    for qt in range(QT):
                        o_sb = osb.tile([P, HW], F32, tag="o", name="o")
                        nc.vector.tensor_mul(
                            out=o_sb[:],
                            in0=pos_[qt][:],
                            in1=rec[:, qt : qt + 1].to_broadcast([P, HW]),
                        )
                        nc.sync.dma_start(
                            out=out[qh * QW + qt * P : qh * QW + (qt + 1) * P, 0:HW],
                            in_=o_sb[:],
                        )

                    # hh1 chains (reuse es, second half of V)
                    pos1 = [
                        ops.tile([P, HW], F32, tag="po", name="po") for _ in range(QT)
                    ]
                    for ki, (c, r, ktl) in enumerate(kts):
                        vt = vp2.tile([P, HW], BF, tag="v2", name="v2")
                        nc.sync.dma_start(out=vt[:], in_=v_all[c][r, ktl][:, HW:H])
                        for qt in range(QT):
                            nc.tensor.matmul(
                                pos1[qt][:],
                                lhsT=es[ki][:, qt * P : (qt + 1) * P],
                                rhs=vt[:],
                                start=(ki == 0),
                                stop=(ki == KT - 1),
                            )
                    for qt in range(QT):
                        o_sb = osb.tile([P, HW], F32, tag="o", name="o")
                        nc.vector.tensor_mul(
                            out=o_sb[:],
                            in0=pos1[qt][:],
                            in1=rec[:, qt : qt + 1].to_broadcast([P, HW]),
                        )
                        nc.sync.dma_start(
                            out=out[qh * QW + qt * P : qh * QW + (qt + 1) * P, HW:H],
                            in_=o_sb[:],
                        )
    nc.finalize()
    return nc


def _prep_inputs(inputs):
    ids = np.asarray(inputs["input_ids"]).astype(np.int32)
    pids = np.asarray(inputs["pos_ids"]).astype(np.int32)
    emb = np.asarray(inputs["emb"], dtype=np.float32).astype(ml_dtypes.bfloat16)
    pemb = np.asarray(inputs["pos_emb"], dtype=np.float32).astype(ml_dtypes.bfloat16)
    W = np.asarray(inputs["W"], dtype=np.float32)
    b = np.asarray(inputs["b"], dtype=np.float32)
    # L is carried x4 on device (fp8 dynamic range); exact powers of two
    wt4 = (W.T * 4.0).astype(ml_dtypes.bfloat16)  # [2H, H]
    wt = np.ascontiguousarray(wt4.reshape(2 * HT, P, H).transpose(1, 0, 2))
    brow = (b * 4.0).reshape(1, H).astype(ml_dtypes.bfloat16)
    in_maps = []
    for i in range(NCORES):
        sl = slice(i * NL, (i + 1) * NL)
        oh = np.zeros((POS, NL), dtype=ml_dtypes.bfloat16)
        oh[pids[sl], np.arange(NL)] = 1.0
        in_maps.append(
            {
                "ids": np.ascontiguousarray(ids[sl].reshape(LRT, P, 1)),
                "emb": emb,
                "pemb": pemb,
                "wt": wt,
                "oh": oh,
                "brow": brow,
            }
        )
    return in_maps


def run(inputs, trace=False):
    nc = build_nc()
    in_maps = _prep_inputs(inputs)
    res = run_bass_kernel_spmd(nc, in_maps, list(range(NCORES)), trace=trace)
    out = np.concatenate([res.results[i]["out"] for i in range(NCORES)], axis=0)
    return out, res


def kernel(**inputs):
    out, _ = run(inputs, trace=False)
    return out


# revision 20
# speedup vs baseline: 1.3059x; 1.0197x over previous
"""Trainium2 Bass kernel for nn_AttentiveEncoderPOS (embed+concat+linear+self-attention).

Sequence-parallel strategy (8 cores, SPMD, AllGather):
  - Core i owns rows i*1024:(i+1)*1024. It gathers ONLY its own 1024 embedding
    rows (bf16 table), computes L_i = concat(emb[ids], pos[pids]) @ W.T + b for
    its slice in BOTH layouts (L.T with h on partitions for the score matmuls,
    and natural V for the AV matmuls), then all-gathers both layouts so every
    core holds the full 8192-row K/V.
  - The position-embedding term folds into the linear as a 64-wide one-hot
    contraction (PW = pos_emb @ W2.T computed on device once); the bias is a
    rank-1 matmul term.  No pos gather, no replicated 8192-row linear.
  - Attention per core: queries = own 1024 rows (SBUF-resident from phase 1),
    keys/values stream from the all-gathered DRAM buffers.  exp(scores) tiles
    for a 512-query half stay in SBUF; AV accumulates over all 64 key tiles in
    single PSUM chains (no SBUF partial-sum accumulation on the vector engine).
  - Scores are tiny (|s|<0.05) so exp() without max-subtraction is exact
    softmax; bf16 matmul inputs, fp32 PSUM accumulation.
  - The AllGathers are chunked (lt/v x first/second half of each rank's key
    tiles) so communication overlaps the start of the attention phase.
"""

import numpy as np
import ml_dtypes

import concourse.bass as bass
import concourse.mybir as mybir
from concourse import bacc
from concourse.tile import TileContext
from concourse.bass_utils import run_bass_kernel_spmd
from concourse.masks import make_identity

N = 8192
H = 1024
VOCAB = 50257
POS = 64
NCORES = 8
NL = N // NCORES          # 1024 rows per core
P = 128
HT = H // P               # 8 h tiles
LRT = NL // P             # 8 local row tiles
KTL = LRT                 # key tiles per rank slab
NCH = 2                   # AG chunks per tensor (kt-local 0:4 / 4:8)
KPC = KTL // NCH          # 4 kt per chunk
KT = N // P               # 64 key tiles global
QH = 2                    # query halves
QW = NL // QH             # 512 queries per half
QT = QW // P              # 4 q tiles per half
HH = 2                    # output h halves
HW = H // HH              # 512
SCALE = 1.0 / 512.0       # 1/sqrt(H) / 16 (L carried x4 for fp8 range)
LAG = 6                   # AV chain lag behind the score/exp pipeline

BF = mybir.dt.bfloat16
F32 = mybir.dt.float32
I32 = mybir.dt.int32
EXP = mybir.ActivationFunctionType.Exp
F8 = mybir.dt.float8e4


def build_nc():
    nc = bacc.Bacc(num_devices=NCORES)
    ids = nc.declare_dram_parameter("ids", [LRT, P, 1], I32, isOutput=False)
    emb = nc.declare_dram_parameter("emb", [VOCAB, H], BF, isOutput=False)
    pemb = nc.declare_dram_parameter("pemb", [POS, H], BF, isOutput=False)
    wt8 = nc.declare_dram_parameter("wt8", [P, HT, H], F8, isOutput=False)  # W1.T swizzled fp8
    wt2 = nc.declare_dram_parameter("wt2", [P, HT, H], BF, isOutput=False)  # 4*W2.T swizzled
    oh = nc.declare_dram_parameter("oh", [POS, NL], BF, isOutput=False)  # onehot.T
    brow = nc.declare_dram_parameter("brow", [1, H], BF, isOutput=False)
    out = nc.declare_dram_parameter("out", [NL, H], F32, isOutput=True)

    # AllGather chunk buffers.  lt slab layout: [ktl, p(h in ht), ht, c(key)]
    # so a key tile loads with 2KB-contiguous partition rows; v is natural.
    lt_din = [nc.dram_tensor(f"lt_din{c}", [KPC, P, HT, P], F8) for c in range(NCH)]
    v_din = [nc.dram_tensor(f"v_din{c}", [KPC, P, H], F8) for c in range(NCH)]
    lt_all = [
        nc.dram_tensor(f"lt_all{c}", [NCORES, KPC, P, HT, P], F8, addr_space="Shared")
        for c in range(NCH)
    ]
    v_all = [
        nc.dram_tensor(f"v_all{c}", [NCORES, KPC, P, H], F8, addr_space="Shared")
        for c in range(NCH)
    ]

    with TileContext(nc) as tc:
        with (
            tc.tile_pool(name="const", bufs=1) as const,
            tc.tile_pool(name="qtp", bufs=HT) as qtp,
        ):
            ident = const.tile([P, P], BF)
            make_identity(nc, ident[:])
            ident32 = const.tile([P, P], F32)
            make_identity(nc, ident32[:])
            ident8 = const.tile([P, P], F8)
            make_identity(nc, ident8[:])
            ones_row = const.tile([1, NL], BF)
            nc.gpsimd.memset(ones_row[:], 1.0)
            fours_col = const.tile([P, 1], BF)
            nc.gpsimd.memset(fours_col[:], 4.0)
            b_row = const.tile([1, H], BF)
            nc.sync.dma_start(out=b_row[:], in_=brow[:, :])
            oh_sb = const.tile([POS, NL], BF)
            nc.sync.dma_start(out=oh_sb[:], in_=oh[:, :])
            pw = const.tile([POS, H], BF)
            qts8 = const.tile([P, HT, NL], F8)
            # query-side L.T (own rows), kept resident for phase 2
            qts = [qtp.tile([P, NL], BF, tag="q", name="q") for _ in range(HT)]

            # ---------------- Phase 1: own-slice linear ----------------
            with (
                tc.tile_pool(name="wtp", bufs=1) as wtp,
                tc.tile_pool(name="idp", bufs=LRT) as idp,
                tc.tile_pool(name="xfp", bufs=LRT) as xfp,
                tc.tile_pool(name="xtp", bufs=1) as xtp,
                tc.tile_pool(name="pep", bufs=HT + 1) as pep,
                tc.tile_pool(name="vsp", bufs=LRT) as vsp,
                tc.tile_pool(name="tps", bufs=2, space="PSUM") as tps,
                tc.tile_pool(name="mps", bufs=3, space="PSUM") as mps,
                tc.tile_pool(name="pps", bufs=2, space="PSUM") as pps,
            ):
                # own embedding row gather first (SWDGE; overlaps the W loads)
                xfs = []
                for rt in range(LRT):
                    idt = idp.tile([P, 1], I32, tag="id", name="id")
                    nc.sync.dma_start(out=idt[:], in_=ids[rt])
                    xf = xfp.tile([P, H], BF, tag="xf", name="xf")
                    nc.gpsimd.indirect_dma_start(
                        out=xf[:],
                        out_offset=None,
                        in_=emb[:],
                        in_offset=bass.IndirectOffsetOnAxis(ap=idt[:, :1], axis=0),
                    )
                    xfs.append(xf)

                # W1 (fp8, linear) and 4*W2 (bf16, PW) -> SBUF
                wt8_all = wtp.tile([P, HT, H], F8, name="wt8all")
                nc.scalar.dma_start(out=wt8_all[:], in_=wt8[:, :, :])
                wt2_all = wtp.tile([P, HT, H], BF, name="wt2all")
                nc.scalar.dma_start(out=wt2_all[:], in_=wt2[:, :, :])

                # transpose gathered rows to X.T
                # xts free layout: k*NL + rt*P + rr
                xts = xtp.tile([P, HT * NL], F8, name="xts")
                for rt in range(LRT):
                    pt = tps.tile([P, H], BF, tag="xt", name="xt")
                    for k in range(HT):
                        nc.tensor.transpose(
                            pt[:, k * P : (k + 1) * P],
                            xfs[rt][:, k * P : (k + 1) * P],
                            ident[:],
                        )
                    nc.vector.tensor_copy(
                        out=xts[:]
                        .rearrange("p (k rt rr) -> p k rt rr", k=HT, rt=LRT)[
                            :, :, rt, :
                        ],
                        in_=pt[:].rearrange("p (k rr) -> p k rr", k=HT),
                    )

                # PW = pemb @ W2.T  (pos table projected once)
                pe_sb = pep.tile([POS, H], BF, tag="pe", name="pe")
                nc.sync.dma_start(out=pe_sb[:], in_=pemb[:, :])
                pets = []
                for jt in range(HT):
                    pt = tps.tile([P, POS], BF, tag="xt", name="pet")
                    nc.tensor.transpose(
                        pt[:], pe_sb[:, jt * P : (jt + 1) * P], ident[0:POS, 0:POS]
                    )
                    pe_t = pep.tile([P, POS], BF, tag="pett", name="pett")
                    nc.vector.tensor_copy(out=pe_t[:], in_=pt[:])
                    pets.append(pe_t)
                for hh in range(HH):
                    pm = pps.tile([POS, HW], F32, tag="pwp", name="pwp")
                    for jt in range(HT):
                        nc.tensor.matmul(
                            pm[:],
                            lhsT=pets[jt][:],
                            rhs=wt2_all[:, jt, hh * HW : (hh + 1) * HW],
                            start=(jt == 0),
                            stop=(jt == HT - 1),
                        )
                    nc.vector.tensor_copy(out=pw[:, hh * HW : (hh + 1) * HW], in_=pm[:])

                def lt_chunk(ch):
                    for ht in range(HT):
                        pm = mps.tile([P, QW], F32, tag="mm", name="mm")
                        xts3 = xts[:].rearrange("p (k r) -> p k r", k=HT)
                        for t in range(HT // 2):
                            nc.tensor.matmul(
                                pm[:],
                                lhsT=wt8_all[:, 2 * t : 2 * t + 2, ht * P : (ht + 1) * P],
                                rhs=xts3[:, 2 * t : 2 * t + 2, ch * QW : (ch + 1) * QW],
                                start=(t == 0),
                                stop=False,
                                perf_mode=mybir.MatmulPerfMode.DoubleRow,
                            )
                        nc.tensor.matmul(
                            pm[:],
                            lhsT=pw[:, ht * P : (ht + 1) * P],
                            rhs=oh_sb[:, ch * QW : (ch + 1) * QW],
                            start=False,
                            stop=False,
                        )
                        nc.tensor.matmul(
                            pm[:],
                            lhsT=b_row[0:1, ht * P : (ht + 1) * P],
                            rhs=ones_row[0:1, ch * QW : (ch + 1) * QW],
                            start=False,
                            stop=True,
                        )
                        nc.vector.tensor_copy(
                            out=qts[ht][:, ch * QW : (ch + 1) * QW], in_=pm[:]
                        )
                        nc.vector.tensor_copy(
                            out=qts8[:, ht, ch * QW : (ch + 1) * QW], in_=pm[:]
                        )
                        nc.sync.dma_start(
                            out=lt_din[ch][:, :, ht, :].rearrange("kb p c -> p kb c"),
                            in_=qts8[:, ht, ch * QW : (ch + 1) * QW].rearrange(
                                "p (kb c) -> p kb c", kb=KPC
                            ),
                        )

                def v_chunk(ch):
                    # V rows = transposes of the finished L.T columns
                    for rt in range(ch * KPC, (ch + 1) * KPC):
                        pt = tps.tile([P, H], BF, tag="xt", name="vt")
                        for ht in range(HT):
                            nc.tensor.transpose(
                                pt[:, ht * P : (ht + 1) * P],
                                qts[ht][:, rt * P : (rt + 1) * P],
                                ident[:],
                            )
                        v_sb = vsp.tile([P, H], F8, tag="vsb", name="vsb")
                        nc.vector.tensor_copy(out=v_sb[:], in_=pt[:])
                        nc.sync.dma_start(out=v_din[rt // KPC][rt % KPC], in_=v_sb[:])

                lt_chunk(0)
                v_chunk(0)
                lt_chunk(1)
                v_chunk(1)

            # ---------------- AllGathers (chunked) ----------------
            for c in range(NCH):
                nc.gpsimd.collective_compute(
                    "AllGather",
                    mybir.AluOpType.bypass,
                    replica_groups=[list(range(NCORES))],
                    ins=[lt_din[c][:, :, :, :]],
                    outs=[lt_all[c][:, :, :, :, :]],
                )
                nc.gpsimd.collective_compute(
                    "AllGather",
                    mybir.AluOpType.bypass,
                    replica_groups=[list(range(NCORES))],
                    ins=[v_din[c][:, :, :]],
                    outs=[v_all[c][:, :, :, :]],
                )

            # ---------------- Phase 2: attention ----------------
            kts = [(c, r, k) for c in range(NCH) for r in range(NCORES) for k in range(KPC)]
            with (
                tc.tile_pool(name="lkp", bufs=6) as lkp,
                tc.tile_pool(name="vp2", bufs=LAG + 4) as vp2,
                tc.tile_pool(name="ep", bufs=KT) as ep,
                tc.tile_pool(name="fin", bufs=4) as fin,
                tc.tile_pool(name="osb", bufs=4) as osb,
                tc.tile_pool(name="sps", bufs=2, space="PSUM") as sps,
                tc.tile_pool(name="ops", bufs=QT, space="PSUM") as ops,
                tc.tile_pool(name="cps", bufs=1, space="PSUM") as cps,
                tc.tile_pool(name="rps", bufs=1, space="PSUM") as rps,
            ):
                rec_ps = rps.tile([P, QH * QT], F32, name="recp")
                for qh in range(QH):
                    qsl = slice(qh * QW, (qh + 1) * QW)
                    psum_cs = cps.tile([1, QW], F32, tag="cs", name="cs")
                    es = []
                    vts = []
                    pos_ = [
                        ops.tile([P, HW], F32, tag="po", name="po") for _ in range(QT)
                    ]

                    def av0(ki):
                        nc.tensor.matmul(
                            psum_cs[0:1, :],
                            lhsT=fours_col[:],
                            rhs=es[ki][:],
                            start=(ki == 0),
                            stop=(ki == KT - 1),
                        )
                        for qt in range(QT):
                            nc.tensor.matmul(
                                pos_[qt][:],
                                lhsT=es[ki][:, qt * P : (qt + 1) * P],
                                rhs=vts[ki][:],
                                start=(ki == 0),
                                stop=(ki == KT - 1),
                            )

                    # stage A: scores+exp; colsum/AV-hh0 run one kt behind so
                    # the exp latency hides under the next score chain
                    for ki, (c, r, ktl) in enumerate(kts):
                        ltk = lkp.tile([P, HT * P], F8, tag="lk", name="lk")
                        nc.sync.dma_start(
                            out=ltk[:].rearrange("p (h c) -> p h c", h=HT),
                            in_=lt_all[c][r, ktl],
                        )
                        ltk3 = ltk[:].rearrange("p (h c) -> p h c", h=HT)
                        ps = sps.tile([P, QW], F32, tag="sp", name="sp")
                        for t in range(HT // 2):
                            nc.tensor.matmul(
                                ps[:],
                                lhsT=ltk3[:, 2 * t : 2 * t + 2, :],
                                rhs=qts8[:, 2 * t : 2 * t + 2, qsl],
                                start=(t == 0),
                                stop=(t == HT // 2 - 1),
                                perf_mode=mybir.MatmulPerfMode.DoubleRow,
                            )
                        e = ep.tile([P, QW], BF, tag="e", name="e")
                        nc.scalar.activation(out=e[:], in_=ps[:], func=EXP, scale=SCALE)
                        es.append(e)
                        vt = vp2.tile([P, HW], F8, tag="v2", name="v2")
                        nc.sync.dma_start(out=vt[:], in_=v_all[c][r, ktl][:, 0:HW])
                        vts.append(vt)
                        if ki >= LAG:
                            av0(ki - LAG)
                    for j in range(KT - LAG, KT):
                        av0(j)

                    # denominator -> reciprocal per q tile
                    cs_sb = fin.tile([1, QW], F32, tag="cs_sb", name="cs_sb")
                    nc.vector.tensor_copy(out=cs_sb[:], in_=psum_cs[0:1, :])
                    for qt in range(QT):
                        nc.tensor.transpose(
                            rec_ps[:, qh * QT + qt : qh * QT + qt + 1],
                            cs_sb[0:1, qt * P : (qt + 1) * P],
                            ident32[0:1, 0:1],
                        )
                    rec = fin.tile([P, QT], F32, tag="rec", name="rec")
                    nc.vector.reciprocal(rec[:], rec_ps[:, qh * QT : (qh + 1) * QT])

                    # finalize hh0
                    for qt in range(QT):
                        o_sb = osb.tile([P, HW], F32, tag="o", name="o")
                        nc.vector.tensor_mul(
                            out=o_sb[:],
                            in0=pos_[qt][:],
                            in1=rec[:, qt : qt + 1].to_broadcast([P, HW]),
                        )
                        nc.sync.dma_start(
                            out=out[qh * QW + qt * P : qh * QW + (qt + 1) * P, 0:HW],
                            in_=o_sb[:],
                        )

                    # hh1 chains (reuse es, second half of V)
                    pos1 = [
                        ops.tile([P, HW], F32, tag="po", name="po") for _ in range(QT)
                    ]
                    for ki, (c, r, ktl) in enumerate(kts):
                        vt = vp2.tile([P, HW], F8, tag="v2", name="v2")
                        nc.sync.dma_start(out=vt[:], in_=v_all[c][r, ktl][:, HW:H])
                        for qt in range(QT):
                            nc.tensor.matmul(
                                pos1[qt][:],
                                lhsT=es[ki][:, qt * P : (qt + 1) * P],
                                rhs=vt[:],
                                start=(ki == 0),
                                stop=(ki == KT - 1),
                            )
                    for qt in range(QT):
                        o_sb = osb.tile([P, HW], F32, tag="o", name="o")
                        nc.vector.tensor_mul(
                            out=o_sb[:],
                            in0=pos1[qt][:],
                            in1=rec[:, qt : qt + 1].to_broadcast([P, HW]),
                        )
                        nc.sync.dma_start(
                            out=out[qh * QW + qt * P : qh * QW + (qt + 1) * P, HW:H],
                            in_=o_sb[:],
                        )
    nc.finalize()
    return nc


def _prep_inputs(inputs):
    ids = np.asarray(inputs["input_ids"]).astype(np.int32)
    pids = np.asarray(inputs["pos_ids"]).astype(np.int32)
    emb = (np.asarray(inputs["emb"], dtype=np.float32) * 4.0).astype(ml_dtypes.bfloat16)
    pemb = np.asarray(inputs["pos_emb"], dtype=np.float32).astype(ml_dtypes.bfloat16)
    W = np.asarray(inputs["W"], dtype=np.float32)
    b = np.asarray(inputs["b"], dtype=np.float32)
    # L is carried x4 on device (fp8 dynamic range): emb x4 in fp8,
    # W1 at natural scale in fp8, 4*W2 in bf16 (PW path), 4*b in bf16.
    F8NP = ml_dtypes.float8_e4m3
    wt8 = np.ascontiguousarray(
        W[:, :H].T.reshape(HT, P, H).transpose(1, 0, 2).astype(F8NP)
    )
    wt2 = np.ascontiguousarray(
        (W[:, H:].T * 4.0).reshape(HT, P, H).transpose(1, 0, 2).astype(ml_dtypes.bfloat16)
    )
    brow = (b * 4.0).reshape(1, H).astype(ml_dtypes.bfloat16)
    in_maps = []
    for i in range(NCORES):
        sl = slice(i * NL, (i + 1) * NL)
        oh = np.zeros((POS, NL), dtype=ml_dtypes.bfloat16)
        oh[pids[sl], np.arange(NL)] = 1.0
        in_maps.append(
            {
                "ids": np.ascontiguousarray(ids[sl].reshape(LRT, P, 1)),
                "emb": emb,
                "pemb": pemb,
                "wt8": wt8,
                "wt2": wt2,
                "oh": oh,
                "brow": brow,
            }
        )
    return in_maps


def run(inputs, trace=False):
    nc = build_nc()
    in_maps = _prep_inputs(inputs)
    res = run_bass_kernel_spmd(nc, in_maps, list(range(NCORES)), trace=trace)
    out = np.concatenate([res.results[i]["out"] for i in range(NCORES)], axis=0)
    return out, res


def kernel(**inputs):
    out, _ = run(inputs, trace=False)
    return out


# revision 21
# speedup vs baseline: 1.3429x; 1.0283x over previous
"""Trainium2 Bass kernel for nn_AttentiveEncoderPOS (embed+concat+linear+self-attention).

Sequence-parallel strategy (8 cores, SPMD, AllGather):
  - Core i owns rows i*1024:(i+1)*1024. It gathers ONLY its own 1024 embedding
    rows (bf16 table), computes L_i = concat(emb[ids], pos[pids]) @ W.T + b for
    its slice in BOTH layouts (L.T with h on partitions for the score matmuls,
    and natural V for the AV matmuls), then all-gathers both layouts so every
    core holds the full 8192-row K/V.
  - The position-embedding term folds into the linear as a 64-wide one-hot
    contraction (PW = pos_emb @ W2.T computed on device once); the bias is a
    rank-1 matmul term.  No pos gather, no replicated 8192-row linear.
  - Attention per core: queries = own 1024 rows (SBUF-resident from phase 1),
    keys/values stream from the all-gathered DRAM buffers.  exp(scores) tiles
    for a 512-query half stay in SBUF; AV accumulates over all 64 key tiles in
    single PSUM chains (no SBUF partial-sum accumulation on the vector engine).
  - Scores are tiny (|s|<0.05) so exp() without max-subtraction is exact
    softmax; bf16 matmul inputs, fp32 PSUM accumulation.
  - The AllGathers are chunked (lt/v x first/second half of each rank's key
    tiles) so communication overlaps the start of the attention phase.
"""

import numpy as np
import ml_dtypes

import concourse.bass as bass
import concourse.mybir as mybir
from concourse import bacc
from concourse.tile import TileContext
from concourse.bass_utils import run_bass_kernel_spmd
from concourse.masks import make_identity

N = 8192
H = 1024
VOCAB = 50257
POS = 64
NCORES = 8
NL = N // NCORES          # 1024 rows per core
P = 128
HT = H // P               # 8 h tiles
LRT = NL // P             # 8 local row tiles
KTL = LRT                 # key tiles per rank slab
NCH = 2                   # AG chunks per tensor (kt-local 0:4 / 4:8)
KPC = KTL // NCH          # 4 kt per chunk
KT = N // P               # 64 key tiles global
QH = 2                    # query halves
QW = NL // QH             # 512 queries per half
QT = QW // P              # 4 q tiles per half
HH = 2                    # output h halves
HW = H // HH              # 512
SCALE = 1.0 / 512.0       # 1/sqrt(H) / 16 (L carried x4 for fp8 range)
LAG = 6                   # AV chain lag behind the score/exp pipeline

BF = mybir.dt.bfloat16
F32 = mybir.dt.float32
I32 = mybir.dt.int32
EXP = mybir.ActivationFunctionType.Exp
F8 = mybir.dt.float8e4


def build_nc():
    nc = bacc.Bacc(num_devices=NCORES)
    ids = nc.declare_dram_parameter("ids", [LRT, P, 1], I32, isOutput=False)
    emb = nc.declare_dram_parameter("emb", [VOCAB, H], BF, isOutput=False)
    pemb = nc.declare_dram_parameter("pemb", [POS, H], BF, isOutput=False)
    wt8 = nc.declare_dram_parameter("wt8", [P, HT, H], F8, isOutput=False)  # W1.T swizzled fp8
    wt2 = nc.declare_dram_parameter("wt2", [P, HT, H], BF, isOutput=False)  # 4*W2.T swizzled
    oh = nc.declare_dram_parameter("oh", [POS, NL], BF, isOutput=False)  # onehot.T
    brow = nc.declare_dram_parameter("brow", [1, H], BF, isOutput=False)
    identp = nc.declare_dram_parameter("identp", [P, P], BF, isOutput=False)
    identp32 = nc.declare_dram_parameter("identp32", [P, P], F32, isOutput=False)
    onesp = nc.declare_dram_parameter("onesp", [1, NL], BF, isOutput=False)
    out = nc.declare_dram_parameter("out", [NL, H], F32, isOutput=True)

    # AllGather chunk buffers.  lt slab layout: [ktl, p(h in ht), ht, c(key)]
    # so a key tile loads with 2KB-contiguous partition rows; v is natural.
    lt_din = [nc.dram_tensor(f"lt_din{c}", [KPC, P, HT, P], F8) for c in range(NCH)]
    v_din = [nc.dram_tensor(f"v_din{c}", [KPC, P, H], F8) for c in range(NCH)]
    lt_all = [
        nc.dram_tensor(f"lt_all{c}", [NCORES, KPC, P, HT, P], F8, addr_space="Shared")
        for c in range(NCH)
    ]
    v_all = [
        nc.dram_tensor(f"v_all{c}", [NCORES, KPC, P, H], F8, addr_space="Shared")
        for c in range(NCH)
    ]
    warm_in = nc.dram_tensor("warm_in", [1, P], BF)
    warm_out = nc.dram_tensor("warm_out", [NCORES, P], BF, addr_space="Shared")

    with TileContext(nc) as tc:
        with (
            tc.tile_pool(name="const", bufs=1) as const,
            tc.tile_pool(name="qtp", bufs=HT) as qtp,
        ):
            ident = const.tile([P, P], BF)
            nc.sync.dma_start(out=ident[:], in_=identp[:, :])
            ident32 = const.tile([P, P], F32)
            nc.sync.dma_start(out=ident32[:], in_=identp32[:, :])
            ones_row = const.tile([1, NL], BF)
            nc.sync.dma_start(out=ones_row[:], in_=onesp[:, :])
            fours_col = const.tile([P, 1], BF)
            nc.scalar.activation(
                out=fours_col[:],
                in_=ident[:, 0:1],
                func=mybir.ActivationFunctionType.Copy,
                scale=0.0,
                bias=4.0,
            )
            b_row = const.tile([1, H], BF)
            nc.sync.dma_start(out=b_row[:], in_=brow[:, :])
            oh_sb = const.tile([POS, NL], BF)
            nc.sync.dma_start(out=oh_sb[:], in_=oh[:, :])
            pw = const.tile([POS, H], BF)
            qts8 = const.tile([P, HT, NL], F8)
            # query-side L.T (own rows), kept resident for phase 2
            qts = [qtp.tile([P, NL], BF, tag="q", name="q") for _ in range(HT)]

            # ---------------- Phase 1: own-slice linear ----------------
            with (
                tc.tile_pool(name="wtp", bufs=1) as wtp,
                tc.tile_pool(name="idp", bufs=LRT) as idp,
                tc.tile_pool(name="xfp", bufs=LRT) as xfp,
                tc.tile_pool(name="xtp", bufs=1) as xtp,
                tc.tile_pool(name="pep", bufs=HT + 1) as pep,
                tc.tile_pool(name="vsp", bufs=LRT) as vsp,
                tc.tile_pool(name="tps", bufs=2, space="PSUM") as tps,
                tc.tile_pool(name="mps", bufs=3, space="PSUM") as mps,
                tc.tile_pool(name="pps", bufs=2, space="PSUM") as pps,
            ):
                # own embedding row gather first (SWDGE; overlaps the W loads)
                xfs = []
                for rt in range(LRT):
                    idt = idp.tile([P, 1], I32, tag="id", name="id")
                    nc.sync.dma_start(out=idt[:], in_=ids[rt])
                    xf = xfp.tile([P, H], BF, tag="xf", name="xf")
                    nc.gpsimd.indirect_dma_start(
                        out=xf[:],
                        out_offset=None,
                        in_=emb[:],
                        in_offset=bass.IndirectOffsetOnAxis(ap=idt[:, :1], axis=0),
                    )
                    xfs.append(xf)

                # warm-up collective: absorbs the first-collective rendezvous
                # cost while phase 1 computes (content unused)
                warm_sb = idp.tile([1, P], BF, tag="warm", name="warm")
                nc.vector.tensor_copy(out=warm_sb[:], in_=ident[0:1, :])
                nc.sync.dma_start(out=warm_in[:, :], in_=warm_sb[:])
                nc.gpsimd.collective_compute(
                    "AllGather",
                    mybir.AluOpType.bypass,
                    replica_groups=[list(range(NCORES))],
                    ins=[warm_in[:, :]],
                    outs=[warm_out[:, :]],
                )

                # W1 (fp8, linear) and 4*W2 (bf16, PW) -> SBUF
                wt8_all = wtp.tile([P, HT, H], F8, name="wt8all")
                nc.scalar.dma_start(out=wt8_all[:], in_=wt8[:, :, :])
                wt2_all = wtp.tile([P, HT, H], BF, name="wt2all")
                nc.scalar.dma_start(out=wt2_all[:], in_=wt2[:, :, :])

                # transpose gathered rows to X.T
                # xts free layout: k*NL + rt*P + rr
                xts = xtp.tile([P, HT * NL], F8, name="xts")
                for rt in range(LRT):
                    pt = tps.tile([P, H], BF, tag="xt", name="xt")
                    for k in range(HT):
                        nc.tensor.transpose(
                            pt[:, k * P : (k + 1) * P],
                            xfs[rt][:, k * P : (k + 1) * P],
                            ident[:],
                        )
                    nc.vector.tensor_copy(
                        out=xts[:]
                        .rearrange("p (k rt rr) -> p k rt rr", k=HT, rt=LRT)[
                            :, :, rt, :
                        ],
                        in_=pt[:].rearrange("p (k rr) -> p k rr", k=HT),
                    )

                # PW = pemb @ W2.T  (pos table projected once)
                pe_sb = pep.tile([POS, H], BF, tag="pe", name="pe")
                nc.sync.dma_start(out=pe_sb[:], in_=pemb[:, :])
                pets = []
                for jt in range(HT):
                    pt = tps.tile([P, POS], BF, tag="xt", name="pet")
                    nc.tensor.transpose(
                        pt[:], pe_sb[:, jt * P : (jt + 1) * P], ident[0:POS, 0:POS]
                    )
                    pe_t = pep.tile([P, POS], BF, tag="pett", name="pett")
                    nc.vector.tensor_copy(out=pe_t[:], in_=pt[:])
                    pets.append(pe_t)
                for hh in range(HH):
                    pm = pps.tile([POS, HW], F32, tag="pwp", name="pwp")
                    for jt in range(HT):
                        nc.tensor.matmul(
                            pm[:],
                            lhsT=pets[jt][:],
                            rhs=wt2_all[:, jt, hh * HW : (hh + 1) * HW],
                            start=(jt == 0),
                            stop=(jt == HT - 1),
                        )
                    nc.vector.tensor_copy(out=pw[:, hh * HW : (hh + 1) * HW], in_=pm[:])

                def lt_chunk(ch):
                    for ht in range(HT):
                        pm = mps.tile([P, QW], F32, tag="mm", name="mm")
                        xts3 = xts[:].rearrange("p (k r) -> p k r", k=HT)
                        for t in range(HT // 2):
                            nc.tensor.matmul(
                                pm[:],
                                lhsT=wt8_all[:, 2 * t : 2 * t + 2, ht * P : (ht + 1) * P],
                                rhs=xts3[:, 2 * t : 2 * t + 2, ch * QW : (ch + 1) * QW],
                                start=(t == 0),
                                stop=False,
                                perf_mode=mybir.MatmulPerfMode.DoubleRow,
                            )
                        nc.tensor.matmul(
                            pm[:],
                            lhsT=pw[:, ht * P : (ht + 1) * P],
                            rhs=oh_sb[:, ch * QW : (ch + 1) * QW],
                            start=False,
                            stop=False,
                        )
                        nc.tensor.matmul(
                            pm[:],
                            lhsT=b_row[0:1, ht * P : (ht + 1) * P],
                            rhs=ones_row[0:1, ch * QW : (ch + 1) * QW],
                            start=False,
                            stop=True,
                        )
                        nc.vector.tensor_copy(
                            out=qts[ht][:, ch * QW : (ch + 1) * QW], in_=pm[:]
                        )
                        nc.vector.tensor_copy(
                            out=qts8[:, ht, ch * QW : (ch + 1) * QW], in_=pm[:]
                        )
                        nc.sync.dma_start(
                            out=lt_din[ch][:, :, ht, :].rearrange("kb p c -> p kb c"),
                            in_=qts8[:, ht, ch * QW : (ch + 1) * QW].rearrange(
                                "p (kb c) -> p kb c", kb=KPC
                            ),
                        )

                def v_chunk(ch):
                    # V rows = transposes of the finished L.T columns
                    for rt in range(ch * KPC, (ch + 1) * KPC):
                        pt = tps.tile([P, H], BF, tag="xt", name="vt")
                        for ht in range(HT):
                            nc.tensor.transpose(
                                pt[:, ht * P : (ht + 1) * P],
                                qts[ht][:, rt * P : (rt + 1) * P],
                                ident[:],
                            )
                        v_sb = vsp.tile([P, H], F8, tag="vsb", name="vsb")
                        nc.vector.tensor_copy(out=v_sb[:], in_=pt[:])
                        nc.sync.dma_start(out=v_din[rt // KPC][rt % KPC], in_=v_sb[:])

                lt_chunk(0)
                v_chunk(0)
                lt_chunk(1)
                v_chunk(1)

            # ---------------- AllGathers (chunked) ----------------
            for c in range(NCH):
                nc.gpsimd.collective_compute(
                    "AllGather",
                    mybir.AluOpType.bypass,
                    replica_groups=[list(range(NCORES))],
                    ins=[lt_din[c][:, :, :, :]],
                    outs=[lt_all[c][:, :, :, :, :]],
                )
                nc.gpsimd.collective_compute(
                    "AllGather",
                    mybir.AluOpType.bypass,
                    replica_groups=[list(range(NCORES))],
                    ins=[v_din[c][:, :, :]],
                    outs=[v_all[c][:, :, :, :]],
                )

            # ---------------- Phase 2: attention ----------------
            kts = [(c, r, k) for c in range(NCH) for r in range(NCORES) for k in range(KPC)]
            with (
                tc.tile_pool(name="lkp", bufs=6) as lkp,
                tc.tile_pool(name="vp2", bufs=LAG + 4) as vp2,
                tc.tile_pool(name="ep", bufs=KT) as ep,
                tc.tile_pool(name="fin", bufs=4) as fin,
                tc.tile_pool(name="osb", bufs=4) as osb,
                tc.tile_pool(name="sps", bufs=2, space="PSUM") as sps,
                tc.tile_pool(name="ops", bufs=QT, space="PSUM") as ops,
                tc.tile_pool(name="cps", bufs=1, space="PSUM") as cps,
                tc.tile_pool(name="rps", bufs=1, space="PSUM") as rps,
            ):
                rec_ps = rps.tile([P, QH * QT], F32, name="recp")
                for qh in range(QH):
                    qsl = slice(qh * QW, (qh + 1) * QW)
                    psum_cs = cps.tile([1, QW], F32, tag="cs", name="cs")
                    es = []
                    vts = []
                    pos_ = [
                        ops.tile([P, HW], F32, tag="po", name="po") for _ in range(QT)
                    ]

                    def av0(ki):
                        nc.tensor.matmul(
                            psum_cs[0:1, :],
                            lhsT=fours_col[:],
                            rhs=es[ki][:],
                            start=(ki == 0),
                            stop=(ki == KT - 1),
                        )
                        for qt in range(QT):
                            nc.tensor.matmul(
                                pos_[qt][:],
                                lhsT=es[ki][:, qt * P : (qt + 1) * P],
                                rhs=vts[ki][:],
                                start=(ki == 0),
                                stop=(ki == KT - 1),
                            )

                    # stage A: scores+exp; colsum/AV-hh0 run one kt behind so
                    # the exp latency hides under the next score chain
                    for ki, (c, r, ktl) in enumerate(kts):
                        ltk = lkp.tile([P, HT * P], F8, tag="lk", name="lk")
                        nc.sync.dma_start(
                            out=ltk[:].rearrange("p (h c) -> p h c", h=HT),
                            in_=lt_all[c][r, ktl],
                        )
                        ltk3 = ltk[:].rearrange("p (h c) -> p h c", h=HT)
                        ps = sps.tile([P, QW], F32, tag="sp", name="sp")
                        for t in range(HT // 2):
                            nc.tensor.matmul(
                                ps[:],
                                lhsT=ltk3[:, 2 * t : 2 * t + 2, :],
                                rhs=qts8[:, 2 * t : 2 * t + 2, qsl],
                                start=(t == 0),
                                stop=(t == HT // 2 - 1),
                                perf_mode=mybir.MatmulPerfMode.DoubleRow,
                            )
                        e = ep.tile([P, QW], BF, tag="e", name="e")
                        nc.scalar.activation(out=e[:], in_=ps[:], func=EXP, scale=SCALE)
                        es.append(e)
                        vt = vp2.tile([P, HW], F8, tag="v2", name="v2")
                        nc.sync.dma_start(out=vt[:], in_=v_all[c][r, ktl][:, 0:HW])
                        vts.append(vt)
                        if ki >= LAG:
                            av0(ki - LAG)
                    for j in range(KT - LAG, KT):
                        av0(j)

                    # denominator -> reciprocal per q tile
                    cs_sb = fin.tile([1, QW], F32, tag="cs_sb", name="cs_sb")
                    nc.vector.tensor_copy(out=cs_sb[:], in_=psum_cs[0:1, :])
                    for qt in range(QT):
                        nc.tensor.transpose(
                            rec_ps[:, qh * QT + qt : qh * QT + qt + 1],
                            cs_sb[0:1, qt * P : (qt + 1) * P],
                            ident32[0:1, 0:1],
                        )
                    rec = fin.tile([P, QT], F32, tag="rec", name="rec")
                    nc.vector.reciprocal(rec[:], rec_ps[:, qh * QT : (qh + 1) * QT])

                    # finalize hh0
                    for qt in range(QT):
                        o_sb = osb.tile([P, HW], F32, tag="o", name="o")
                        nc.vector.tensor_mul(
                            out=o_sb[:],
                            in0=pos_[qt][:],
                            in1=rec[:, qt : qt + 1].to_broadcast([P, HW]),
                        )
                        nc.sync.dma_start(
                            out=out[qh * QW + qt * P : qh * QW + (qt + 1) * P, 0:HW],
                            in_=o_sb[:],
                        )

                    # hh1 chains (reuse es, second half of V)
                    pos1 = [
                        ops.tile([P, HW], F32, tag="po", name="po") for _ in range(QT)
                    ]
                    for ki, (c, r, ktl) in enumerate(kts):
                        vt = vp2.tile([P, HW], F8, tag="v2", name="v2")
                        nc.sync.dma_start(out=vt[:], in_=v_all[c][r, ktl][:, HW:H])
                        for qt in range(QT):
                            nc.tensor.matmul(
                                pos1[qt][:],
                                lhsT=es[ki][:, qt * P : (qt + 1) * P],
                                rhs=vt[:],
                                start=(ki == 0),
                                stop=(ki == KT - 1),
                            )
                    for qt in range(QT):
                        o_sb = osb.tile([P, HW], F32, tag="o", name="o")
                        nc.vector.tensor_mul(
                            out=o_sb[:],
                            in0=pos1[qt][:],
                            in1=rec[:, qt : qt + 1].to_broadcast([P, HW]),
                        )
                        nc.sync.dma_start(
                            out=out[qh * QW + qt * P : qh * QW + (qt + 1) * P, HW:H],
                            in_=o_sb[:],
                        )
    nc.finalize()
    return nc


def _prep_inputs(inputs):
    ids = np.asarray(inputs["input_ids"]).astype(np.int32)
    pids = np.asarray(inputs["pos_ids"]).astype(np.int32)
    emb = (np.asarray(inputs["emb"], dtype=np.float32) * 4.0).astype(ml_dtypes.bfloat16)
    pemb = np.asarray(inputs["pos_emb"], dtype=np.float32).astype(ml_dtypes.bfloat16)
    W = np.asarray(inputs["W"], dtype=np.float32)
    b = np.asarray(inputs["b"], dtype=np.float32)
    # L is carried x4 on device (fp8 dynamic range): emb x4 in fp8,
    # W1 at natural scale in fp8, 4*W2 in bf16 (PW path), 4*b in bf16.
    F8NP = ml_dtypes.float8_e4m3
    wt8 = np.ascontiguousarray(
        W[:, :H].T.reshape(HT, P, H).transpose(1, 0, 2).astype(F8NP)
    )
    wt2 = np.ascontiguousarray(
        (W[:, H:].T * 4.0).reshape(HT, P, H).transpose(1, 0, 2).astype(ml_dtypes.bfloat16)
    )
    brow = (b * 4.0).reshape(1, H).astype(ml_dtypes.bfloat16)
    identp = np.eye(P, dtype=np.float32).astype(ml_dtypes.bfloat16)
    identp32 = np.eye(P, dtype=np.float32)
    onesp = np.ones((1, NL), dtype=ml_dtypes.bfloat16)
    in_maps = []
    for i in range(NCORES):
        sl = slice(i * NL, (i + 1) * NL)
        oh = np.zeros((POS, NL), dtype=ml_dtypes.bfloat16)
        oh[pids[sl], np.arange(NL)] = 1.0
        in_maps.append(
            {
                "ids": np.ascontiguousarray(ids[sl].reshape(LRT, P, 1)),
                "emb": emb,
                "pemb": pemb,
                "wt8": wt8,
                "wt2": wt2,
                "oh": oh,
                "brow": brow,
                "identp": identp,
                "identp32": identp32,
                "onesp": onesp,
            }
        )
    return in_maps


def run(inputs, trace=False):
    nc = build_nc()
    in_maps = _prep_inputs(inputs)
    res = run_bass_kernel_spmd(nc, in_maps, list(range(NCORES)), trace=trace)
    out = np.concatenate([res.results[i]["out"] for i in range(NCORES)], axis=0)
    return out, res


def kernel(**inputs):
    out, _ = run(inputs, trace=False)
    return out


# revision 23
# speedup vs baseline: 1.3919x; 1.0365x over previous
"""Trainium2 Bass kernel for nn_AttentiveEncoderPOS (embed+concat+linear+self-attention).

Sequence-parallel strategy (8 cores, SPMD, AllGather):
  - Core i owns rows i*1024:(i+1)*1024. It gathers ONLY its own 1024 embedding
    rows (bf16 table), computes L_i = concat(emb[ids], pos[pids]) @ W.T + b for
    its slice in BOTH layouts (L.T with h on partitions for the score matmuls,
    and natural V for the AV matmuls), then all-gathers both layouts so every
    core holds the full 8192-row K/V.
  - The position-embedding term folds into the linear as a 64-wide one-hot
    contraction (PW = pos_emb @ W2.T computed on device once); the bias is a
    rank-1 matmul term.  No pos gather, no replicated 8192-row linear.
  - Attention per core: queries = own 1024 rows (SBUF-resident from phase 1),
    keys/values stream from the all-gathered DRAM buffers.  exp(scores) tiles
    for a 512-query half stay in SBUF; AV accumulates over all 64 key tiles in
    single PSUM chains (no SBUF partial-sum accumulation on the vector engine).
  - Scores are tiny (|s|<0.05) so exp() without max-subtraction is exact
    softmax; bf16 matmul inputs, fp32 PSUM accumulation.
  - The AllGathers are chunked (lt/v x first/second half of each rank's key
    tiles) so communication overlaps the start of the attention phase.
"""

import numpy as np
import ml_dtypes

import concourse.bass as bass
import concourse.mybir as mybir
from concourse import bacc
from concourse.tile import TileContext
from concourse.bass_utils import run_bass_kernel_spmd
from concourse.masks import make_identity

N = 8192
H = 1024
VOCAB = 50257
POS = 64
NCORES = 8
NL = N // NCORES          # 1024 rows per core
P = 128
HT = H // P               # 8 h tiles
LRT = NL // P             # 8 local row tiles
KTL = LRT                 # key tiles per rank slab
NCH = 2                   # AG chunks per tensor (kt-local 0:4 / 4:8)
KPC = KTL // NCH          # 4 kt per chunk
KT = N // P               # 64 key tiles global
QH = 2                    # query halves
QW = NL // QH             # 512 queries per half
QT = QW // P              # 4 q tiles per half
HH = 2                    # output h halves
HW = H // HH              # 512
SCALE = 1.0 / 512.0       # 1/sqrt(H) / 16 (L carried x4 for fp8 range)
LAG = 6                   # AV chain lag behind the score/exp pipeline

BF = mybir.dt.bfloat16
F32 = mybir.dt.float32
I32 = mybir.dt.int32
EXP = mybir.ActivationFunctionType.Exp
F8 = mybir.dt.float8e4


def build_nc():
    nc = bacc.Bacc(num_devices=NCORES)
    ids = nc.declare_dram_parameter("ids", [LRT, P, 1], I32, isOutput=False)
    emb = nc.declare_dram_parameter("emb", [VOCAB, H], BF, isOutput=False)
    pemb = nc.declare_dram_parameter("pemb", [POS, H], BF, isOutput=False)
    wt8 = nc.declare_dram_parameter("wt8", [P, HT, H], F8, isOutput=False)  # W1.T swizzled fp8
    wt2 = nc.declare_dram_parameter("wt2", [P, HT, H], BF, isOutput=False)  # 4*W2.T swizzled
    oh = nc.declare_dram_parameter("oh", [POS, NL], BF, isOutput=False)  # onehot.T
    brow = nc.declare_dram_parameter("brow", [1, H], BF, isOutput=False)
    identp = nc.declare_dram_parameter("identp", [P, P], BF, isOutput=False)
    identp32 = nc.declare_dram_parameter("identp32", [P, P], F32, isOutput=False)
    onesp = nc.declare_dram_parameter("onesp", [1, NL], BF, isOutput=False)
    out = nc.declare_dram_parameter("out", [NL, H], F32, isOutput=True)

    # AllGather chunk buffers.  lt slab layout: [ktl, p(h in ht), ht, c(key)]
    # so a key tile loads with 2KB-contiguous partition rows; v is natural.
    lt_din = [nc.dram_tensor(f"lt_din{c}", [KPC, P, HT, P], F8) for c in range(NCH)]
    v_din = [nc.dram_tensor(f"v_din{c}", [KPC, P, H], F8) for c in range(NCH)]
    lt_all = [
        nc.dram_tensor(f"lt_all{c}", [NCORES, KPC, P, HT, P], F8, addr_space="Shared")
        for c in range(NCH)
    ]
    v_all = [
        nc.dram_tensor(f"v_all{c}", [NCORES, KPC, P, H], F8, addr_space="Shared")
        for c in range(NCH)
    ]
    warm_in = nc.dram_tensor("warm_in", [1, P], BF)
    warm_out = nc.dram_tensor("warm_out", [NCORES, P], BF, addr_space="Shared")

    with TileContext(nc) as tc:
        with (
            tc.tile_pool(name="const", bufs=1) as const,
            tc.tile_pool(name="qtp", bufs=HT) as qtp,
        ):
            ident = const.tile([P, P], BF)
            nc.sync.dma_start(out=ident[:], in_=identp[:, :])
            ident32 = const.tile([P, P], F32)
            nc.sync.dma_start(out=ident32[:], in_=identp32[:, :])
            ones_row = const.tile([1, NL], BF)
            nc.sync.dma_start(out=ones_row[:], in_=onesp[:, :])
            fours_col = const.tile([P, 1], BF)
            nc.scalar.activation(
                out=fours_col[:],
                in_=ident[:, 0:1],
                func=mybir.ActivationFunctionType.Copy,
                scale=0.0,
                bias=4.0,
            )
            b_row = const.tile([1, H], BF)
            nc.sync.dma_start(out=b_row[:], in_=brow[:, :])
            oh_sb = const.tile([POS, NL], BF)
            nc.sync.dma_start(out=oh_sb[:], in_=oh[:, :])
            pw = const.tile([POS, H], BF)
            qts8 = const.tile([P, HT, NL], F8)
            # query-side L.T (own rows), kept resident for phase 2
            qts = [qtp.tile([P, NL], BF, tag="q", name="q") for _ in range(HT)]

            # ---------------- Phase 1: own-slice linear ----------------
            with (
                tc.tile_pool(name="wtp", bufs=1) as wtp,
                tc.tile_pool(name="idp", bufs=LRT) as idp,
                tc.tile_pool(name="xfp", bufs=LRT) as xfp,
                tc.tile_pool(name="xtp", bufs=1) as xtp,
                tc.tile_pool(name="pep", bufs=HT + 1) as pep,
                tc.tile_pool(name="vsp", bufs=LRT) as vsp,
                tc.tile_pool(name="tps", bufs=2, space="PSUM") as tps,
                tc.tile_pool(name="mps", bufs=3, space="PSUM") as mps,
                tc.tile_pool(name="pps", bufs=2, space="PSUM") as pps,
            ):
                # own embedding row gather first (SWDGE; overlaps the W loads)
                xfs = []
                for rt in range(LRT):
                    idt = idp.tile([P, 1], I32, tag="id", name="id")
                    nc.sync.dma_start(out=idt[:], in_=ids[rt])
                    xf = xfp.tile([P, H], BF, tag="xf", name="xf")
                    nc.gpsimd.indirect_dma_start(
                        out=xf[:],
                        out_offset=None,
                        in_=emb[:],
                        in_offset=bass.IndirectOffsetOnAxis(ap=idt[:, :1], axis=0),
                    )
                    xfs.append(xf)

                # warm-up collective: absorbs the first-collective rendezvous
                # cost while phase 1 computes (content unused)
                warm_sb = idp.tile([1, P], BF, tag="warm", name="warm")
                nc.vector.tensor_copy(out=warm_sb[:], in_=ident[0:1, :])
                nc.sync.dma_start(out=warm_in[:, :], in_=warm_sb[:])
                nc.gpsimd.collective_compute(
                    "AllGather",
                    mybir.AluOpType.bypass,
                    replica_groups=[list(range(NCORES))],
                    ins=[warm_in[:, :]],
                    outs=[warm_out[:, :]],
                )

                # W1 (fp8, linear) and 4*W2 (bf16, PW) -> SBUF
                wt8_all = wtp.tile([P, HT, H], F8, name="wt8all")
                nc.scalar.dma_start(out=wt8_all[:], in_=wt8[:, :, :])
                wt2_all = wtp.tile([P, HT, H], BF, name="wt2all")
                nc.scalar.dma_start(out=wt2_all[:], in_=wt2[:, :, :])

                # transpose gathered rows to X.T
                # xts free layout: k*NL + rt*P + rr
                xts = xtp.tile([P, HT * NL], F8, name="xts")
                for rt in range(LRT):
                    pt = tps.tile([P, H], BF, tag="xt", name="xt")
                    for k in range(HT):
                        nc.tensor.transpose(
                            pt[:, k * P : (k + 1) * P],
                            xfs[rt][:, k * P : (k + 1) * P],
                            ident[:],
                        )
                    nc.vector.tensor_copy(
                        out=xts[:]
                        .rearrange("p (k rt rr) -> p k rt rr", k=HT, rt=LRT)[
                            :, :, rt, :
                        ],
                        in_=pt[:].rearrange("p (k rr) -> p k rr", k=HT),
                    )

                # PW = pemb @ W2.T  (pos table projected once)
                pe_sb = pep.tile([POS, H], BF, tag="pe", name="pe")
                nc.sync.dma_start(out=pe_sb[:], in_=pemb[:, :])
                pets = []
                for jt in range(HT):
                    pt = tps.tile([P, POS], BF, tag="xt", name="pet")
                    nc.tensor.transpose(
                        pt[:], pe_sb[:, jt * P : (jt + 1) * P], ident[0:POS, 0:POS]
                    )
                    pe_t = pep.tile([P, POS], BF, tag="pett", name="pett")
                    nc.vector.tensor_copy(out=pe_t[:], in_=pt[:])
                    pets.append(pe_t)
                for hh in range(HH):
                    pm = pps.tile([POS, HW], F32, tag="pwp", name="pwp")
                    for jt in range(HT):
                        nc.tensor.matmul(
                            pm[:],
                            lhsT=pets[jt][:],
                            rhs=wt2_all[:, jt, hh * HW : (hh + 1) * HW],
                            start=(jt == 0),
                            stop=(jt == HT - 1),
                        )
                    nc.vector.tensor_copy(out=pw[:, hh * HW : (hh + 1) * HW], in_=pm[:])

                def lt_chunk(ch):
                    for ht in range(HT):
                        pm = mps.tile([P, QW], F32, tag="mm", name="mm")
                        xts3 = xts[:].rearrange("p (k r) -> p k r", k=HT)
                        for t in range(HT // 2):
                            nc.tensor.matmul(
                                pm[:],
                                lhsT=wt8_all[:, 2 * t : 2 * t + 2, ht * P : (ht + 1) * P],
                                rhs=xts3[:, 2 * t : 2 * t + 2, ch * QW : (ch + 1) * QW],
                                start=(t == 0),
                                stop=False,
                                perf_mode=mybir.MatmulPerfMode.DoubleRow,
                            )
                        nc.tensor.matmul(
                            pm[:],
                            lhsT=pw[:, ht * P : (ht + 1) * P],
                            rhs=oh_sb[:, ch * QW : (ch + 1) * QW],
                            start=False,
                            stop=False,
                        )
                        nc.tensor.matmul(
                            pm[:],
                            lhsT=b_row[0:1, ht * P : (ht + 1) * P],
                            rhs=ones_row[0:1, ch * QW : (ch + 1) * QW],
                            start=False,
                            stop=True,
                        )
                        nc.vector.tensor_copy(
                            out=qts[ht][:, ch * QW : (ch + 1) * QW], in_=pm[:]
                        )
                        nc.vector.tensor_copy(
                            out=qts8[:, ht, ch * QW : (ch + 1) * QW], in_=pm[:]
                        )
                        nc.sync.dma_start(
                            out=lt_din[ch][:, :, ht, :].rearrange("kb p c -> p kb c"),
                            in_=qts8[:, ht, ch * QW : (ch + 1) * QW].rearrange(
                                "p (kb c) -> p kb c", kb=KPC
                            ),
                        )

                def v_chunk(ch):
                    # V rows = transposes of the finished L.T columns
                    for rt in range(ch * KPC, (ch + 1) * KPC):
                        pt = tps.tile([P, H], BF, tag="xt", name="vt")
                        for ht in range(HT):
                            nc.tensor.transpose(
                                pt[:, ht * P : (ht + 1) * P],
                                qts[ht][:, rt * P : (rt + 1) * P],
                                ident[:],
                            )
                        v_sb = vsp.tile([P, H], F8, tag="vsb", name="vsb")
                        nc.vector.tensor_copy(out=v_sb[:], in_=pt[:])
                        nc.sync.dma_start(out=v_din[rt // KPC][rt % KPC], in_=v_sb[:])

                lt_chunk(0)
                v_chunk(0)
                lt_chunk(1)
                v_chunk(1)

            # ---------------- AllGathers (chunked) ----------------
            for c in range(NCH):
                nc.gpsimd.collective_compute(
                    "AllGather",
                    mybir.AluOpType.bypass,
                    replica_groups=[list(range(NCORES))],
                    ins=[lt_din[c][:, :, :, :]],
                    outs=[lt_all[c][:, :, :, :, :]],
                )
                nc.gpsimd.collective_compute(
                    "AllGather",
                    mybir.AluOpType.bypass,
                    replica_groups=[list(range(NCORES))],
                    ins=[v_din[c][:, :, :]],
                    outs=[v_all[c][:, :, :, :]],
                )

            # ---------------- Phase 2: attention ----------------
            kts = [(c, r, k) for c in range(NCH) for r in range(NCORES) for k in range(KPC)]
            with (
                tc.tile_pool(name="lkp", bufs=KT // NCH + 2) as lkp,
                tc.tile_pool(name="vp2", bufs=LAG + 4) as vp2,
                tc.tile_pool(name="ep", bufs=KT) as ep,
                tc.tile_pool(name="fin", bufs=4) as fin,
                tc.tile_pool(name="osb", bufs=4) as osb,
                tc.tile_pool(name="sps", bufs=2, space="PSUM") as sps,
                tc.tile_pool(name="ops", bufs=QT, space="PSUM") as ops,
                tc.tile_pool(name="cps", bufs=1, space="PSUM") as cps,
                tc.tile_pool(name="rps", bufs=1, space="PSUM") as rps,
            ):
                rec_ps = rps.tile([P, QH * QT], F32, name="recp")
                for qh in range(QH):
                    qsl = slice(qh * QW, (qh + 1) * QW)
                    psum_cs = cps.tile([1, QW], F32, tag="cs", name="cs")
                    es = []
                    vts = []
                    pos_ = [
                        ops.tile([P, HW], F32, tag="po", name="po") for _ in range(QT)
                    ]

                    def av0(ki):
                        nc.tensor.matmul(
                            psum_cs[0:1, :],
                            lhsT=fours_col[:],
                            rhs=es[ki][:],
                            start=(ki == 0),
                            stop=(ki == KT - 1),
                        )
                        for qt in range(QT):
                            nc.tensor.matmul(
                                pos_[qt][:],
                                lhsT=es[ki][:, qt * P : (qt + 1) * P],
                                rhs=vts[ki][:],
                                start=(ki == 0),
                                stop=(ki == KT - 1),
                            )

                    # stage A: scores+exp; colsum/AV-hh0 run LAG kts behind
                    # so exp latency and v arrival hide under the score stream.
                    # All of a chunk's key tiles are DMA'd before any v tile so
                    # v-waits never block the score pipeline on the sync queue.
                    ltks = {}
                    for ki, (c, r, ktl) in enumerate(kts):
                        if ki % (KT // NCH) == 0:
                            for kj in range(ki, ki + KT // NCH):
                                cc_, rr_, kk_ = kts[kj]
                                lt_t = lkp.tile([P, HT * P], F8, tag="lk", name="lk")
                                nc.sync.dma_start(
                                    out=lt_t[:].rearrange("p (h c) -> p h c", h=HT),
                                    in_=lt_all[cc_][rr_, kk_],
                                )
                                ltks[kj] = lt_t
                        ltk3 = ltks[ki][:].rearrange("p (h c) -> p h c", h=HT)
                        ps = sps.tile([P, QW], F32, tag="sp", name="sp")
                        for t in range(HT // 2):
                            nc.tensor.matmul(
                                ps[:],
                                lhsT=ltk3[:, 2 * t : 2 * t + 2, :],
                                rhs=qts8[:, 2 * t : 2 * t + 2, qsl],
                                start=(t == 0),
                                stop=(t == HT // 2 - 1),
                                perf_mode=mybir.MatmulPerfMode.DoubleRow,
                            )
                        e = ep.tile([P, QW], BF, tag="e", name="e")
                        nc.scalar.activation(out=e[:], in_=ps[:], func=EXP, scale=SCALE)
                        es.append(e)
                        vt = vp2.tile([P, HW], F8, tag="v2", name="v2")
                        nc.sync.dma_start(out=vt[:], in_=v_all[c][r, ktl][:, 0:HW])
                        vts.append(vt)
                        if ki >= LAG:
                            av0(ki - LAG)
                    for j in range(KT - LAG, KT):
                        av0(j)

                    # denominator -> reciprocal per q tile
                    cs_sb = fin.tile([1, QW], F32, tag="cs_sb", name="cs_sb")
                    nc.vector.tensor_copy(out=cs_sb[:], in_=psum_cs[0:1, :])
                    for qt in range(QT):
                        nc.tensor.transpose(
                            rec_ps[:, qh * QT + qt : qh * QT + qt + 1],
                            cs_sb[0:1, qt * P : (qt + 1) * P],
                            ident32[0:1, 0:1],
                        )
                    rec = fin.tile([P, QT], F32, tag="rec", name="rec")
                    nc.vector.reciprocal(rec[:], rec_ps[:, qh * QT : (qh + 1) * QT])

                    # finalize hh0
                    for qt in range(QT):
                        o_sb = osb.tile([P, HW], F32, tag="o", name="o")
                        nc.vector.tensor_mul(
                            out=o_sb[:],
                            in0=pos_[qt][:],
                            in1=rec[:, qt : qt + 1].to_broadcast([P, HW]),
                        )
                        nc.sync.dma_start(
                            out=out[qh * QW + qt * P : qh * QW + (qt + 1) * P, 0:HW],
                            in_=o_sb[:],
                        )

                    # hh1 chains (reuse es, second half of V)
                    pos1 = [
                        ops.tile([P, HW], F32, tag="po", name="po") for _ in range(QT)
                    ]
                    for ki, (c, r, ktl) in enumerate(kts):
                        vt = vp2.tile([P, HW], F8, tag="v2", name="v2")
                        nc.sync.dma_start(out=vt[:], in_=v_all[c][r, ktl][:, HW:H])
                        for qt in range(QT):
                            nc.tensor.matmul(
                                pos1[qt][:],
                                lhsT=es[ki][:, qt * P : (qt + 1) * P],
                                rhs=vt[:],
                                start=(ki == 0),
                                stop=(ki == KT - 1),
                            )
                    for qt in range(QT):
                        o_sb = osb.tile([P, HW], F32, tag="o", name="o")
                        nc.vector.tensor_mul(
                            out=o_sb[:],
                            in0=pos1[qt][:],
                            in1=rec[:, qt : qt + 1].to_broadcast([P, HW]),
                        )
                        nc.sync.dma_start(
                            out=out[qh * QW + qt * P : qh * QW + (qt + 1) * P, HW:H],
                            in_=o_sb[:],
                        )
    nc.finalize()
    return nc


def _prep_inputs(inputs):
    ids = np.asarray(inputs["input_ids"]).astype(np.int32)
    pids = np.asarray(inputs["pos_ids"]).astype(np.int32)
    emb = (np.asarray(inputs["emb"], dtype=np.float32) * 4.0).astype(ml_dtypes.bfloat16)
    pemb = np.asarray(inputs["pos_emb"], dtype=np.float32).astype(ml_dtypes.bfloat16)
    W = np.asarray(inputs["W"], dtype=np.float32)
    b = np.asarray(inputs["b"], dtype=np.float32)
    # L is carried x4 on device (fp8 dynamic range): emb x4 in fp8,
    # W1 at natural scale in fp8, 4*W2 in bf16 (PW path), 4*b in bf16.
    F8NP = ml_dtypes.float8_e4m3
    wt8 = np.ascontiguousarray(
        W[:, :H].T.reshape(HT, P, H).transpose(1, 0, 2).astype(F8NP)
    )
    wt2 = np.ascontiguousarray(
        (W[:, H:].T * 4.0).reshape(HT, P, H).transpose(1, 0, 2).astype(ml_dtypes.bfloat16)
    )
    brow = (b * 4.0).reshape(1, H).astype(ml_dtypes.bfloat16)
    identp = np.eye(P, dtype=np.float32).astype(ml_dtypes.bfloat16)
    identp32 = np.eye(P, dtype=np.float32)
    onesp = np.ones((1, NL), dtype=ml_dtypes.bfloat16)
    in_maps = []
    for i in range(NCORES):
        sl = slice(i * NL, (i + 1) * NL)
        oh = np.zeros((POS, NL), dtype=ml_dtypes.bfloat16)
        oh[pids[sl], np.arange(NL)] = 1.0
        in_maps.append(
            {
                "ids": np.ascontiguousarray(ids[sl].reshape(LRT, P, 1)),
                "emb": emb,
                "pemb": pemb,
                "wt8": wt8,
                "wt2": wt2,
                "oh": oh,
                "brow": brow,
                "identp": identp,
                "identp32": identp32,
                "onesp": onesp,
            }
        )
    return in_maps


def run(inputs, trace=False):
    nc = build_nc()
    in_maps = _prep_inputs(inputs)
    res = run_bass_kernel_spmd(nc, in_maps, list(range(NCORES)), trace=trace)
    out = np.concatenate([res.results[i]["out"] for i in range(NCORES)], axis=0)
    return out, res


def kernel(**inputs):
    out, _ = run(inputs, trace=False)
    return out


# revision 24
# speedup vs baseline: 1.4661x; 1.0534x over previous
"""Trainium2 Bass kernel for nn_AttentiveEncoderPOS (embed+concat+linear+self-attention).

Sequence-parallel strategy (8 cores, SPMD, AllGather):
  - Core i owns rows i*1024:(i+1)*1024. It gathers ONLY its own 1024 embedding
    rows (bf16 table), computes L_i = concat(emb[ids], pos[pids]) @ W.T + b for
    its slice in BOTH layouts (L.T with h on partitions for the score matmuls,
    and natural V for the AV matmuls), then all-gathers both layouts so every
    core holds the full 8192-row K/V.
  - The position-embedding term folds into the linear as a 64-wide one-hot
    contraction (PW = pos_emb @ W2.T computed on device once); the bias is a
    rank-1 matmul term.  No pos gather, no replicated 8192-row linear.
  - Attention per core: queries = own 1024 rows (SBUF-resident from phase 1),
    keys/values stream from the all-gathered DRAM buffers.  exp(scores) tiles
    for a 512-query half stay in SBUF; AV accumulates over all 64 key tiles in
    single PSUM chains (no SBUF partial-sum accumulation on the vector engine).
  - Scores are tiny (|s|<0.05) so exp() without max-subtraction is exact
    softmax; bf16 matmul inputs, fp32 PSUM accumulation.
  - The AllGathers are chunked (lt/v x first/second half of each rank's key
    tiles) so communication overlaps the start of the attention phase.
"""

import numpy as np
import ml_dtypes

import concourse.bass as bass
import concourse.mybir as mybir
from concourse import bacc
from concourse.tile import TileContext
from concourse.bass_utils import run_bass_kernel_spmd
from concourse.masks import make_identity

N = 8192
H = 1024
VOCAB = 50257
POS = 64
NCORES = 8
NL = N // NCORES          # 1024 rows per core
P = 128
HT = H // P               # 8 h tiles
LRT = NL // P             # 8 local row tiles
KTL = LRT                 # key tiles per rank slab
NCH = 2                   # AG chunks per tensor (kt-local 0:4 / 4:8)
KPC = KTL // NCH          # 4 kt per chunk
KT = N // P               # 64 key tiles global
QH = 2                    # query halves
QW = NL // QH             # 512 queries per half
QT = QW // P              # 4 q tiles per half
HH = 2                    # output h halves
HW = H // HH              # 512
SCALE = 1.0 / 512.0       # 1/sqrt(H) / 16 (L carried x4 for fp8 range)
LAG = 6                   # AV chain lag behind the score/exp pipeline

BF = mybir.dt.bfloat16
F32 = mybir.dt.float32
I32 = mybir.dt.int32
EXP = mybir.ActivationFunctionType.Exp
F8 = mybir.dt.float8e4


def build_nc():
    nc = bacc.Bacc(num_devices=NCORES)
    ids = nc.declare_dram_parameter("ids", [LRT, P, 1], I32, isOutput=False)
    emb = nc.declare_dram_parameter("emb", [VOCAB, H], BF, isOutput=False)
    pemb = nc.declare_dram_parameter("pemb", [POS, H], BF, isOutput=False)
    wt8 = nc.declare_dram_parameter("wt8", [P, HT, H], F8, isOutput=False)  # W1.T swizzled fp8
    wt2 = nc.declare_dram_parameter("wt2", [P, HT, H], BF, isOutput=False)  # 4*W2.T swizzled
    oh = nc.declare_dram_parameter("oh", [POS, NL], BF, isOutput=False)  # onehot.T
    brow = nc.declare_dram_parameter("brow", [1, H], BF, isOutput=False)
    identp = nc.declare_dram_parameter("identp", [P, P], BF, isOutput=False)
    identp32 = nc.declare_dram_parameter("identp32", [P, P], F32, isOutput=False)
    onesp = nc.declare_dram_parameter("onesp", [1, NL], BF, isOutput=False)
    out = nc.declare_dram_parameter("out", [NL, H], F32, isOutput=True)

    # AllGather chunk buffers.  lt slab layout: [ktl, p(h in ht), ht, c(key)]
    # so a key tile loads with 2KB-contiguous partition rows; v is natural.
    lt_din = [nc.dram_tensor(f"lt_din{c}", [KPC, P, HT, P], F8) for c in range(NCH)]
    v_din = [nc.dram_tensor(f"v_din{c}", [KPC, P, H], F8) for c in range(NCH)]
    lt_all = [
        nc.dram_tensor(f"lt_all{c}", [NCORES, KPC, P, HT, P], F8, addr_space="Shared")
        for c in range(NCH)
    ]
    v_all = [
        nc.dram_tensor(f"v_all{c}", [NCORES, KPC, P, H], F8, addr_space="Shared")
        for c in range(NCH)
    ]
    warm_in = nc.dram_tensor("warm_in", [1, P], BF)
    warm_out = nc.dram_tensor("warm_out", [NCORES, P], BF, addr_space="Shared")

    with TileContext(nc) as tc:
        with (
            tc.tile_pool(name="const", bufs=1) as const,
            tc.tile_pool(name="qtp", bufs=HT) as qtp,
        ):
            ident = const.tile([P, P], BF)
            nc.sync.dma_start(out=ident[:], in_=identp[:, :])
            ident32 = const.tile([P, P], F32)
            nc.sync.dma_start(out=ident32[:], in_=identp32[:, :])
            ones_row = const.tile([1, NL], BF)
            nc.sync.dma_start(out=ones_row[:], in_=onesp[:, :])
            fours_col = const.tile([P, 1], F32)
            nc.scalar.activation(
                out=fours_col[:],
                in_=ident32[:, 0:1],
                func=mybir.ActivationFunctionType.Copy,
                scale=0.0,
                bias=4.0,
            )
            b_row = const.tile([1, H], BF)
            nc.sync.dma_start(out=b_row[:], in_=brow[:, :])
            oh_sb = const.tile([POS, NL], BF)
            nc.sync.dma_start(out=oh_sb[:], in_=oh[:, :])
            pw = const.tile([POS, H], BF)
            qts8 = const.tile([P, HT, NL], F8)
            # query-side L.T (own rows), kept resident for phase 2
            qts = [qtp.tile([P, NL], BF, tag="q", name="q") for _ in range(HT)]

            # ---------------- Phase 1: own-slice linear ----------------
            with (
                tc.tile_pool(name="wtp", bufs=1) as wtp,
                tc.tile_pool(name="idp", bufs=LRT) as idp,
                tc.tile_pool(name="xfp", bufs=LRT) as xfp,
                tc.tile_pool(name="xtp", bufs=1) as xtp,
                tc.tile_pool(name="pep", bufs=HT + 1) as pep,
                tc.tile_pool(name="vsp", bufs=LRT) as vsp,
                tc.tile_pool(name="tps", bufs=2, space="PSUM") as tps,
                tc.tile_pool(name="mps", bufs=3, space="PSUM") as mps,
                tc.tile_pool(name="pps", bufs=2, space="PSUM") as pps,
            ):
                # own embedding row gather first (SWDGE; overlaps the W loads)
                xfs = []
                for rt in range(LRT):
                    idt = idp.tile([P, 1], I32, tag="id", name="id")
                    nc.sync.dma_start(out=idt[:], in_=ids[rt])
                    xf = xfp.tile([P, H], BF, tag="xf", name="xf")
                    nc.gpsimd.indirect_dma_start(
                        out=xf[:],
                        out_offset=None,
                        in_=emb[:],
                        in_offset=bass.IndirectOffsetOnAxis(ap=idt[:, :1], axis=0),
                    )
                    xfs.append(xf)

                # warm-up collective: absorbs the first-collective rendezvous
                # cost while phase 1 computes (content unused)
                warm_sb = idp.tile([1, P], BF, tag="warm", name="warm")
                nc.vector.tensor_copy(out=warm_sb[:], in_=ident[0:1, :])
                nc.sync.dma_start(out=warm_in[:, :], in_=warm_sb[:])
                nc.gpsimd.collective_compute(
                    "AllGather",
                    mybir.AluOpType.bypass,
                    replica_groups=[list(range(NCORES))],
                    ins=[warm_in[:, :]],
                    outs=[warm_out[:, :]],
                )

                # W1 (fp8, linear) and 4*W2 (bf16, PW) -> SBUF
                wt8_all = wtp.tile([P, HT, H], F8, name="wt8all")
                nc.scalar.dma_start(out=wt8_all[:], in_=wt8[:, :, :])
                wt2_all = wtp.tile([P, HT, H], BF, name="wt2all")
                nc.scalar.dma_start(out=wt2_all[:], in_=wt2[:, :, :])

                # transpose gathered rows to X.T
                # xts free layout: k*NL + rt*P + rr
                xts = xtp.tile([P, HT * NL], F8, name="xts")
                for rt in range(LRT):
                    pt = tps.tile([P, H], BF, tag="xt", name="xt")
                    for k in range(HT):
                        nc.tensor.transpose(
                            pt[:, k * P : (k + 1) * P],
                            xfs[rt][:, k * P : (k + 1) * P],
                            ident[:],
                        )
                    nc.vector.tensor_copy(
                        out=xts[:]
                        .rearrange("p (k rt rr) -> p k rt rr", k=HT, rt=LRT)[
                            :, :, rt, :
                        ],
                        in_=pt[:].rearrange("p (k rr) -> p k rr", k=HT),
                    )

                # PW = pemb @ W2.T  (pos table projected once)
                pe_sb = pep.tile([POS, H], BF, tag="pe", name="pe")
                nc.sync.dma_start(out=pe_sb[:], in_=pemb[:, :])
                pets = []
                for jt in range(HT):
                    pt = tps.tile([P, POS], BF, tag="xt", name="pet")
                    nc.tensor.transpose(
                        pt[:], pe_sb[:, jt * P : (jt + 1) * P], ident[0:POS, 0:POS]
                    )
                    pe_t = pep.tile([P, POS], BF, tag="pett", name="pett")
                    nc.vector.tensor_copy(out=pe_t[:], in_=pt[:])
                    pets.append(pe_t)
                for hh in range(HH):
                    pm = pps.tile([POS, HW], F32, tag="pwp", name="pwp")
                    for jt in range(HT):
                        nc.tensor.matmul(
                            pm[:],
                            lhsT=pets[jt][:],
                            rhs=wt2_all[:, jt, hh * HW : (hh + 1) * HW],
                            start=(jt == 0),
                            stop=(jt == HT - 1),
                        )
                    nc.vector.tensor_copy(out=pw[:, hh * HW : (hh + 1) * HW], in_=pm[:])

                def lt_chunk(ch):
                    for ht in range(HT):
                        pm = mps.tile([P, QW], F32, tag="mm", name="mm")
                        xts3 = xts[:].rearrange("p (k r) -> p k r", k=HT)
                        for t in range(HT // 2):
                            nc.tensor.matmul(
                                pm[:],
                                lhsT=wt8_all[:, 2 * t : 2 * t + 2, ht * P : (ht + 1) * P],
                                rhs=xts3[:, 2 * t : 2 * t + 2, ch * QW : (ch + 1) * QW],
                                start=(t == 0),
                                stop=False,
                                perf_mode=mybir.MatmulPerfMode.DoubleRow,
                            )
                        nc.tensor.matmul(
                            pm[:],
                            lhsT=pw[:, ht * P : (ht + 1) * P],
                            rhs=oh_sb[:, ch * QW : (ch + 1) * QW],
                            start=False,
                            stop=False,
                        )
                        nc.tensor.matmul(
                            pm[:],
                            lhsT=b_row[0:1, ht * P : (ht + 1) * P],
                            rhs=ones_row[0:1, ch * QW : (ch + 1) * QW],
                            start=False,
                            stop=True,
                        )
                        nc.vector.tensor_copy(
                            out=qts[ht][:, ch * QW : (ch + 1) * QW], in_=pm[:]
                        )
                        nc.vector.tensor_copy(
                            out=qts8[:, ht, ch * QW : (ch + 1) * QW], in_=pm[:]
                        )
                        nc.sync.dma_start(
                            out=lt_din[ch][:, :, ht, :].rearrange("kb p c -> p kb c"),
                            in_=qts8[:, ht, ch * QW : (ch + 1) * QW].rearrange(
                                "p (kb c) -> p kb c", kb=KPC
                            ),
                        )

                def v_chunk(ch):
                    # V rows = transposes of the finished L.T columns
                    for rt in range(ch * KPC, (ch + 1) * KPC):
                        pt = tps.tile([P, H], BF, tag="xt", name="vt")
                        for ht in range(HT):
                            nc.tensor.transpose(
                                pt[:, ht * P : (ht + 1) * P],
                                qts[ht][:, rt * P : (rt + 1) * P],
                                ident[:],
                            )
                        v_sb = vsp.tile([P, H], F8, tag="vsb", name="vsb")
                        nc.vector.tensor_copy(out=v_sb[:], in_=pt[:])
                        nc.sync.dma_start(out=v_din[rt // KPC][rt % KPC], in_=v_sb[:])

                lt_chunk(0)
                v_chunk(0)
                lt_chunk(1)
                v_chunk(1)

            # ---------------- AllGathers (chunked) ----------------
            for c in range(NCH):
                nc.gpsimd.collective_compute(
                    "AllGather",
                    mybir.AluOpType.bypass,
                    replica_groups=[list(range(NCORES))],
                    ins=[lt_din[c][:, :, :, :]],
                    outs=[lt_all[c][:, :, :, :, :]],
                )
                nc.gpsimd.collective_compute(
                    "AllGather",
                    mybir.AluOpType.bypass,
                    replica_groups=[list(range(NCORES))],
                    ins=[v_din[c][:, :, :]],
                    outs=[v_all[c][:, :, :, :]],
                )

            # ---------------- Phase 2: attention ----------------
            kts = [(c, r, k) for c in range(NCH) for r in range(NCORES) for k in range(KPC)]
            with (
                tc.tile_pool(name="lkp", bufs=KT // NCH + 2) as lkp,
                tc.tile_pool(name="vp2", bufs=LAG + 4) as vp2,
                tc.tile_pool(name="ep", bufs=KT) as ep,
                tc.tile_pool(name="fin", bufs=4) as fin,
                tc.tile_pool(name="accp", bufs=2) as accp,
                tc.tile_pool(name="osb", bufs=4) as osb,
                tc.tile_pool(name="sps", bufs=2, space="PSUM") as sps,
                tc.tile_pool(name="ops", bufs=QT, space="PSUM") as ops,
                tc.tile_pool(name="cps", bufs=1, space="PSUM") as cps,
                tc.tile_pool(name="rps", bufs=1, space="PSUM") as rps,
            ):
                rec_ps = rps.tile([P, QH * QT], F32, name="recp")
                for qh in range(QH):
                    qsl = slice(qh * QW, (qh + 1) * QW)
                    psum_cs = cps.tile([1, QW], F32, tag="cs", name="cs")
                    es = []
                    vts = []
                    acc = accp.tile([P, QW], F32, tag="acc", name="acc")
                    pos_ = [
                        ops.tile([P, HW], F32, tag="po", name="po") for _ in range(QT)
                    ]

                    def av0(ki):
                        if ki == 0:
                            nc.vector.tensor_copy(out=acc[:], in_=es[0][:])
                        else:
                            nc.vector.tensor_add(out=acc[:], in0=acc[:], in1=es[ki][:])
                        for qt in range(QT):
                            nc.tensor.matmul(
                                pos_[qt][:],
                                lhsT=es[ki][:, qt * P : (qt + 1) * P],
                                rhs=vts[ki][:],
                                start=(ki == 0),
                                stop=(ki == KT - 1),
                            )

                    # stage A: scores+exp; colsum/AV-hh0 run LAG kts behind
                    # so exp latency and v arrival hide under the score stream.
                    # All of a chunk's key tiles are DMA'd before any v tile so
                    # v-waits never block the score pipeline on the sync queue.
                    ltks = {}
                    for ki, (c, r, ktl) in enumerate(kts):
                        if ki % (KT // NCH) == 0:
                            for kj in range(ki, ki + KT // NCH):
                                cc_, rr_, kk_ = kts[kj]
                                lt_t = lkp.tile([P, HT * P], F8, tag="lk", name="lk")
                                nc.sync.dma_start(
                                    out=lt_t[:].rearrange("p (h c) -> p h c", h=HT),
                                    in_=lt_all[cc_][rr_, kk_],
                                )
                                ltks[kj] = lt_t
                        ltk3 = ltks[ki][:].rearrange("p (h c) -> p h c", h=HT)
                        ps = sps.tile([P, QW], F32, tag="sp", name="sp")
                        for t in range(HT // 2):
                            nc.tensor.matmul(
                                ps[:],
                                lhsT=ltk3[:, 2 * t : 2 * t + 2, :],
                                rhs=qts8[:, 2 * t : 2 * t + 2, qsl],
                                start=(t == 0),
                                stop=(t == HT // 2 - 1),
                                perf_mode=mybir.MatmulPerfMode.DoubleRow,
                            )
                        e = ep.tile([P, QW], BF, tag="e", name="e")
                        nc.scalar.activation(out=e[:], in_=ps[:], func=EXP, scale=SCALE)
                        es.append(e)
                        vt = vp2.tile([P, HW], F8, tag="v2", name="v2")
                        nc.sync.dma_start(out=vt[:], in_=v_all[c][r, ktl][:, 0:HW])
                        vts.append(vt)
                        if ki >= LAG:
                            av0(ki - LAG)
                    for j in range(KT - LAG, KT):
                        av0(j)

                    # denominator: one f32 matmul over the accumulated e
                    nc.tensor.matmul(
                        psum_cs[0:1, :],
                        lhsT=fours_col[:],
                        rhs=acc[:],
                        start=True,
                        stop=True,
                    )
                    cs_sb = fin.tile([1, QW], F32, tag="cs_sb", name="cs_sb")
                    nc.vector.tensor_copy(out=cs_sb[:], in_=psum_cs[0:1, :])
                    for qt in range(QT):
                        nc.tensor.transpose(
                            rec_ps[:, qh * QT + qt : qh * QT + qt + 1],
                            cs_sb[0:1, qt * P : (qt + 1) * P],
                            ident32[0:1, 0:1],
                        )
                    rec = fin.tile([P, QT], F32, tag="rec", name="rec")
                    nc.vector.reciprocal(rec[:], rec_ps[:, qh * QT : (qh + 1) * QT])

                    # finalize hh0
                    for qt in range(QT):
                        o_sb = osb.tile([P, HW], F32, tag="o", name="o")
                        nc.vector.tensor_mul(
                            out=o_sb[:],
                            in0=pos_[qt][:],
                            in1=rec[:, qt : qt + 1].to_broadcast([P, HW]),
                        )
                        nc.sync.dma_start(
                            out=out[qh * QW + qt * P : qh * QW + (qt + 1) * P, 0:HW],
                            in_=o_sb[:],
                        )

                    # hh1 chains (reuse es, second half of V)
                    pos1 = [
                        ops.tile([P, HW], F32, tag="po", name="po") for _ in range(QT)
                    ]
                    for ki, (c, r, ktl) in enumerate(kts):
                        vt = vp2.tile([P, HW], F8, tag="v2", name="v2")
                        nc.sync.dma_start(out=vt[:], in_=v_all[c][r, ktl][:, HW:H])
                        for qt in range(QT):
                            nc.tensor.matmul(
                                pos1[qt][:],
                                lhsT=es[ki][:, qt * P : (qt + 1) * P],
                                rhs=vt[:],
                                start=(ki == 0),
                                stop=(ki == KT - 1),
                            )
                    for qt in range(QT):
                        o_sb = osb.tile([P, HW], F32, tag="o", name="o")
                        nc.vector.tensor_mul(
                            out=o_sb[:],
                            in0=pos1[qt][:],
                            in1=rec[:, qt : qt + 1].to_broadcast([P, HW]),
                        )
                        nc.sync.dma_start(
                            out=out[qh * QW + qt * P : qh * QW + (qt + 1) * P, HW:H],
                            in_=o_sb[:],
                        )
    nc.finalize()
    return nc


def _prep_inputs(inputs):
    ids = np.asarray(inputs["input_ids"]).astype(np.int32)
    pids = np.asarray(inputs["pos_ids"]).astype(np.int32)
    emb = (np.asarray(inputs["emb"], dtype=np.float32) * 4.0).astype(ml_dtypes.bfloat16)
    pemb = np.asarray(inputs["pos_emb"], dtype=np.float32).astype(ml_dtypes.bfloat16)
    W = np.asarray(inputs["W"], dtype=np.float32)
    b = np.asarray(inputs["b"], dtype=np.float32)
    # L is carried x4 on device (fp8 dynamic range): emb x4 in fp8,
    # W1 at natural scale in fp8, 4*W2 in bf16 (PW path), 4*b in bf16.
    F8NP = ml_dtypes.float8_e4m3
    wt8 = np.ascontiguousarray(
        W[:, :H].T.reshape(HT, P, H).transpose(1, 0, 2).astype(F8NP)
    )
    wt2 = np.ascontiguousarray(
        (W[:, H:].T * 4.0).reshape(HT, P, H).transpose(1, 0, 2).astype(ml_dtypes.bfloat16)
    )
    brow = (b * 4.0).reshape(1, H).astype(ml_dtypes.bfloat16)
    identp = np.eye(P, dtype=np.float32).astype(ml_dtypes.bfloat16)
    identp32 = np.eye(P, dtype=np.float32)
    onesp = np.ones((1, NL), dtype=ml_dtypes.bfloat16)
    in_maps = []
    for i in range(NCORES):
        sl = slice(i * NL, (i + 1) * NL)
        oh = np.zeros((POS, NL), dtype=ml_dtypes.bfloat16)
        oh[pids[sl], np.arange(NL)] = 1.0
        in_maps.append(
            {
                "ids": np.ascontiguousarray(ids[sl].reshape(LRT, P, 1)),
                "emb": emb,
                "pemb": pemb,
                "wt8": wt8,
                "wt2": wt2,
                "oh": oh,
                "brow": brow,
                "identp": identp,
                "identp32": identp32,
                "onesp": onesp,
            }
        )
    return in_maps


def run(inputs, trace=False):
    nc = build_nc()
    in_maps = _prep_inputs(inputs)
    res = run_bass_kernel_spmd(nc, in_maps, list(range(NCORES)), trace=trace)
    out = np.concatenate([res.results[i]["out"] for i in range(NCORES)], axis=0)
    return out, res


def kernel(**inputs):
    out, _ = run(inputs, trace=False)
    return out
